# revision 15
# baseline (speedup 1.0000x reference)
"""Trainium2 Bass kernel for nn_BiLSTM_centric_layer — segmented-recurrence design.

Key ideas vs the naive data-parallel kernel:

1. The LSTM recurrence is LATENCY-bound on TRN2: each chain's loop-carried
   dependency (matmul -> tanh(gates) -> cell update -> tanh(C) -> h) costs
   ~2-3us of semaphore/engine-init latency per step, and plain BiLSTM only
   exposes 2 independent chains (fwd/bwd). We exploit exponential LSTM state
   forgetting (prod of sigmoid(f) decays ~e^-0.45/step) to split the sequence
   into P segments, each warmed up W steps from zero state (numerically
   validated: W=16 -> ~3e-4 state error, decaying further downstream).
   This gives 2P independent chains.

2. K=4 segments advance in lockstep inside single fat instructions (one
   tanh over all of them), amortizing the ~370ns fixed cost of each
   Activation-engine op; R=5 such groups rotate to hide the dependency
   latency.

3. Input-gate terms x@Wih are accumulated straight into the gates PSUM tile
   by the PE (3 extra tiny matmuls per gate chunk) — no xg precompute pass,
   no DRAM round trip, no vector-engine add.

4. h' = 2h = (tanh_o + 1) * tanh(C) is ONE fused DVE op
   (scalar_tensor_tensor); the 0.5 factors are folded into Whh, Wq and the
   mean-pool mask on the host. h' is written fp16 directly into the SBUF
   history tensor which the next step's matmul reads in place.

5. Attention scores are computed directly from the history via
   w_tilde = (0.5 Wq) @ k (rank-1), so q is only materialized inside the
   fused q + attn (x) v PSUM accumulation.

Sharding: data-parallel over batch (4 rows/core x 8 cores), weights
replicated. Everything hardcoded for B=32, S_RAW=1024, S_SUM=128, D_IN=300,
H=256, NH=4.
"""
import os
import sys

sys.path.insert(0, "/opt/trn_rl_repo")

import numpy as np
import ml_dtypes

import concourse.bacc as bacc
import concourse.bass as bass
import concourse.mybir as mybir
import concourse.tile as tile
from concourse import bass_utils
from concourse.masks import make_identity

F32 = mybir.dt.float32
F32R = mybir.dt.float32r
FP16 = mybir.dt.float16
AF = mybir.ActivationFunctionType
ALU = mybir.AluOpType

B, S, SS, D_IN, H, NH = 32, 1024, 128, 300, 256, 4
DH = 128
BC = 4             # batch rows per core
NCORES = 8
PAD = 32           # zero padding (timesteps) on both ends of x
XR = S + 2 * PAD   # padded raw length
XS = SS + 2 * PAD  # padded sum length
W = int(os.environ.get("K_W", "16"))            # warmup steps
assert 1 <= W <= PAD

# rotation groups: (base, K segments, seg_len) covering [base, base+K*L);
# fwd seg j = [base+j*L, ...); bwd mirrored. Fat K amortizes the ~36ns
# per-matmul weight-load cost; >=2 raw groups hide the chain latency.
RAW_GROUPS = [(0, 16, 32), (512, 16, 32)]
assert sum(Kg * L for _, Kg, L in RAW_GROUPS) == S
SUM_GROUPS = [(0, 16, 8)]
assert sum(Kg * L for _, Kg, L in SUM_GROUPS) == SS


class Group:
    def __init__(self, gid, base, Kg, L, dirsets, Sg, is_sum):
        self.id, self.base, self.K, self.L = gid, base, Kg, L
        self.dirsets, self.Sg, self.is_sum = dirsets, Sg, is_sum
        self.rounds = W + L


def _tap(t, off, dims):
    full = t[:]
    return bass.AP(tensor=full.tensor, offset=full.offset + off,
                   ap=[list(full.ap[0])] + [list(d) for d in dims])


def build_nc():
    nc = bacc.Bacc("TRN2", target_bir_lowering=False, debug=False)

    # ---- DRAM I/O ----
    xr_d = nc.dram_tensor("xr", [3, 128, BC, XR], FP16, kind="ExternalInput")
    xs_d = nc.dram_tensor("xs", [3, 128, BC, XS], FP16, kind="ExternalInput")
    wih_d, whh_d = {}, {}
    for nm in ["rf", "rb", "sf", "sb"]:
        wih_d[nm] = nc.dram_tensor(f"wih_{nm}", [3, 128, 8, 128], FP16,
                                   kind="ExternalInput")
        whh_d[nm] = nc.dram_tensor(f"whh_{nm}", [2, 128, 8, 128], FP16,
                                   kind="ExternalInput")
    wqst_d = nc.dram_tensor("wqst", [4, 128, NH, DH], FP16, kind="ExternalInput")
    wqtt_d = nc.dram_tensor("wqtt", [128, NH, 4, DH], FP16, kind="ExternalInput")
    wkv_d = nc.dram_tensor("wkv", [128, 2, NH, 4, DH], FP16, kind="ExternalInput")
    maskdiv_d = nc.dram_tensor("maskdiv", [BC, SS], FP16, kind="ExternalInput")
    out_d = nc.dram_tensor("out", [BC, S, NH * DH], F32, kind="ExternalOutput")

    with tile.TileContext(nc) as tc:
        persist = tc.alloc_tile_pool(name="persist", bufs=1)
        rec = tc.alloc_tile_pool(name="rec", bufs=1)        # released before E
        work = tc.alloc_tile_pool(name="work", bufs=2)
        gps = tc.alloc_tile_pool(name="gps", bufs=1, space="PSUM")

        ident = persist.tile([128, 128], F32, tag="ident", name="ident")
        make_identity(nc, ident[:])
        identh = persist.tile([128, 128], FP16, tag="identh", name="identh")
        nc.vector.tensor_copy(identh[:], ident[:])

        # ---- stage inputs in SBUF ----
        xr_sb = rec.tile([128, 3, BC, XR], FP16, tag="xr", name="xr_sb")
        nc.sync.dma_start(xr_sb[:], xr_d[:].rearrange("kc p b t -> p kc b t"))
        xs_sb = rec.tile([128, 3, BC, XS], FP16, tag="xs", name="xs_sb")
        nc.sync.dma_start(xs_sb[:], xs_d[:].rearrange("kc p b t -> p kc b t"))
        wih, whh = {}, {}
        for nm in ["rf", "rb", "sf", "sb"]:
            wih[nm] = rec.tile([128, 3, 8, 128], FP16, tag=f"wih{nm}", name=f"wih{nm}")
            nc.sync.dma_start(wih[nm][:], wih_d[nm][:].rearrange("kc p mc c -> p kc mc c"))
            whh[nm] = rec.tile([128, 2, 8, 128], FP16, tag=f"whh{nm}", name=f"whh{nm}")
            nc.sync.dma_start(whh[nm][:], whh_d[nm][:].rearrange("kc p mc c -> p kc mc c"))

        # history tensors (fp16, hold h' = 2h)
        rawT = persist.tile([128, 4, BC, S], FP16, tag="rawT", name="rawT")
        sumT = persist.tile([128, 4, BC, SS], FP16, tag="sumT", name="sumT")

        # ---- group state ----
        groups = []
        for gi, (base, Kg, L) in enumerate(RAW_GROUPS):
            groups.append(Group(gi, base, Kg, L, ("rf", "rb"), S, False))
        for gi, (base, Kg, L) in enumerate(SUM_GROUPS):
            groups.append(Group(len(RAW_GROUPS) + gi, base, Kg, L,
                                ("sf", "sb"), SS, True))

        C, hs = {}, {}
        for g in groups:
            C[g.id] = rec.tile([128, 2, 2, g.K, BC], F32, tag=f"C{g.id}",
                               name=f"C{g.id}")
            nc.vector.memset(C[g.id][:], 0.0)
            hs[g.id] = []
            for par in range(2):
                t = rec.tile([128, 2, 2, g.K, BC], FP16, tag=f"hs{g.id}_{par}",
                             name=f"hs{g.id}_{par}")
                hs[g.id].append(t)

        def xcol0(g, rr, d):
            # x column (into padded buffer) for segment 0 at round rr
            if d == 0:
                return PAD + g.base + (rr - W)
            return PAD + (g.Sg - g.base) + (W - 1) - rr

        def hist_t0(g, rstep, d):
            # history t for segment 0 at real step rstep
            if d == 0:
                return g.base + rstep
            return g.Sg - 1 - g.base - rstep

        def emit_round(g, rr):
            xsb = xs_sb if g.is_sum else xr_sb
            XL = XS if g.is_sum else XR
            hist = sumT if g.is_sum else rawT
            Sg, L, Kg = g.Sg, g.L, g.K
            ps = gps.tile([128, 2, 8, Kg, BC], F32, tag=f"ps{g.id}", name=f"ps{g.id}")
            for d in (0, 1):
                st = L if d == 0 else -L
                c0 = xcol0(g, rr, d)
                wi, wh = wih[g.dirsets[d]], whh[g.dirsets[d]]
                for mc in range(8):
                    o = ps[:, d, mc, :, :]
                    for kc in range(3):
                        mv = _tap(xsb, kc * (BC * XL) + c0, [[st, Kg], [XL, BC]])
                        nc.tensor.matmul(o, wi[:, kc, mc, :], mv,
                                         start=(kc == 0),
                                         stop=(kc == 2 and rr == 0))
                    if rr > 0:
                        for kc in range(2):
                            if rr - 1 < W:
                                hm = hs[g.id][(rr - 1) % 2][:, d, kc, :, :]
                            else:
                                t0p = hist_t0(g, rr - 1 - W, d)
                                hm = _tap(hist, (2 * d + kc) * BC * Sg + t0p,
                                          [[st, Kg], [Sg, BC]])
                            nc.tensor.matmul(o, wh[:, kc, mc, :], hm,
                                             start=False, stop=(kc == 1))
            th = work.tile([128, 2, 8, Kg, BC], FP16, tag=f"th{g.id}", name=f"th{g.id}")
            nc.scalar.activation(th[:], ps[:], AF.Tanh)
            # state D = 2C:  D' = (tf+1)*0.5*D + (ti+1)*tg ; tc = tanh(0.5 D')
            ths = {blk: th[:, :, 2 * blk:2 * blk + 2, :, :]
                   .rearrange("p d m k b -> p d m (k b)") for blk in range(4)}
            Dap = C[g.id][:].rearrange("p d m k b -> p d m (k b)")
            u = work.tile([128, 2, 2, Kg * BC], F32, tag=f"u{g.id}", name=f"u{g.id}")
            nc.vector.scalar_tensor_tensor(out=u[:], in0=ths[0], scalar=1.0,
                                           in1=ths[2], op0=ALU.add, op1=ALU.mult)
            v = work.tile([128, 2, 2, Kg * BC], F32, tag=f"v{g.id}", name=f"v{g.id}")
            nc.vector.scalar_tensor_tensor(out=v[:], in0=ths[1], scalar=1.0,
                                           in1=Dap, op0=ALU.add, op1=ALU.mult)
            nc.vector.scalar_tensor_tensor(out=Dap, in0=v[:], scalar=0.5,
                                           in1=u[:], op0=ALU.mult, op1=ALU.add)
            tcl = work.tile([128, 2, 2, Kg, BC], FP16, tag=f"tc{g.id}", name=f"tc{g.id}")
            nc.scalar.activation(tcl[:], C[g.id][:], AF.Tanh, scale=0.5)
            for d in (0, 1):
                eng = nc.vector
                for kc in range(2):
                    if rr < W:
                        dst = hs[g.id][rr % 2][:, d, kc, :, :]
                    else:
                        st = L if d == 0 else -L
                        t0 = hist_t0(g, rr - W, d)
                        dst = _tap(hist, (2 * d + kc) * BC * Sg + t0,
                                   [[st, Kg], [Sg, BC]])
                    eng.scalar_tensor_tensor(
                        out=dst, in0=th[:, d, 6 + kc, :, :], scalar=1.0,
                        in1=tcl[:, d, kc, :, :], op0=ALU.add, op1=ALU.mult)
            if rr == W - 1 and g.base == 0:
                # segment 0 (fwd [0,L), bwd [Sg-L,Sg)) starts from the true
                # zero state: discard its garbage warmup state.
                nc.vector.memset(C[g.id][:, :, :, 0, :], 0.0)
                nc.vector.memset(hs[g.id][(W - 1) % 2][:, :, :, 0, :], 0.0)

        # phase D tiles that later phases need
        kT_r = persist.tile([128, NH, BC], FP16, tag="kT_r", name="kT_r")
        v1 = persist.tile([1, BC, NH, DH], FP16, tag="v1", name="v1")
        wt_r = persist.tile([128, NH, 4, BC], FP16, tag="wt_r", name="wt_r")

        def emit_phase_d():
            with tc.tile_pool(name="dpool", bufs=1) as pl, \
                 tc.tile_pool(name="d_ps", bufs=1, space="PSUM") as dps:
                msk = pl.tile([128, 4, BC, SS], FP16, tag="msk", name="msk")
                srcap = bass.AP(tensor=maskdiv_d, offset=0,
                                ap=[[0, 128], [SS, BC], [1, SS]])
                for dk in range(4):
                    nc.sync.dma_start(msk[:, dk, :, :], srcap)
                masked = pl.tile([128, 4, BC, SS], FP16, tag="masked", name="masked")
                nc.vector.tensor_tensor(out=masked[:], in0=sumT[:], in1=msk[:],
                                        op=ALU.mult)
                sv = pl.tile([128, 4, BC], F32, tag="sv", name="sv")
                nc.vector.tensor_reduce(out=sv[:], in_=masked[:],
                                        axis=mybir.AxisListType.X, op=ALU.add)
                sv_h = pl.tile([128, 4, BC], FP16, tag="sv_h", name="sv_h")
                nc.vector.tensor_copy(sv_h[:], sv[:])

                wkv = pl.tile([128, 2, NH, 4, DH], FP16, tag="wkv", name="wkv")
                nc.sync.dma_start(wkv[:], wkv_d[:])
                ps_kv = dps.tile([128, NH, 2, BC], F32, tag="dps", name="ps_kv")
                for h in range(NH):
                    for ih in range(2):
                        for dk in range(4):
                            nc.tensor.matmul(ps_kv[:, h, ih, :],
                                             wkv[:, ih, h, dk, :],
                                             sv_h[:, dk, :],
                                             start=(dk == 0), stop=(dk == 3))
                nc.vector.tensor_copy(kT_r[:], ps_kv[:, :, 0, :])
                v_sb = pl.tile([128, NH, BC], FP16, tag="v_sb", name="v_sb")
                nc.scalar.copy(v_sb[:], ps_kv[:, :, 1, :])
                ps_vt = dps.tile([BC, NH, DH], FP16, tag="dps", name="ps_vt")
                for h in range(NH):
                    nc.tensor.transpose(ps_vt[:, h, :], v_sb[:, h, :], identh[:])
                v4 = pl.tile([BC, NH, DH], FP16, tag="v4", name="v4")
                nc.vector.tensor_copy(v4[:], ps_vt[:])
                for b in range(BC):
                    nc.sync.dma_start(v1[:, b, :, :], v4[b:b + 1, :, :])

                # w_tilde[dmod, h, dk, b] = sum_e (0.5 Wq)[h][dk*128+dmod, e] k[e, h, b]
                wqtt = pl.tile([128, NH, 4, DH], FP16, tag="wqtt", name="wqtt")
                nc.sync.dma_start(wqtt[:], wqtt_d[:])
                ps_wt = dps.tile([128, NH, 4, BC], F32, tag="dps", name="ps_wt")
                for h in range(NH):
                    for dk in range(4):
                        nc.tensor.matmul(ps_wt[:, h, dk, :], wqtt[:, h, dk, :],
                                         kT_r[:, h, :], start=True, stop=True)
                nc.vector.tensor_copy(wt_r[:], ps_wt[:])

        # ================= recurrence rotation =================
        max_rounds = max(g.rounds for g in groups)
        sum_last = max(g.rounds for g in groups if g.is_sum) - 1
        for rr in range(max_rounds):
            for g in groups:
                if rr < g.rounds:
                    emit_round(g, rr)
            if rr == sum_last:
                emit_phase_d()

        gps.release()
        work.release()
        rec.release()

        # ================= phase E: scores, softmax, rst =================
        with tc.tile_pool(name="att", bufs=1) as att, \
             tc.tile_pool(name="attw", bufs=2) as attw, \
             tc.tile_pool(name="sc_ps", bufs=3, space="PSUM") as sc_ps, \
             tc.tile_pool(name="r_ps", bufs=3, space="PSUM") as r_ps, \
             tc.tile_pool(name="t_ps", bufs=2, space="PSUM") as t_ps:
            wq_r = att.tile([128, 4, NH, DH], FP16, tag="wq_r", name="wq_r")
            nc.sync.dma_start(wq_r[:], wqst_d[:].rearrange("dk p h e -> p dk h e"))

            # scores[b*4+h, s] = sum_d rawT[d, b, s] * wt[d, h, b]
            scores_sb = att.tile([16, S], F32, tag="scores", name="scores")
            ei = 0
            for b in range(BC):
                for half in range(2):
                    sl = slice(half * 512, (half + 1) * 512)
                    pssc = sc_ps.tile([4, 512], F32, tag="sc", name="sc")
                    for dk in range(4):
                        nc.tensor.matmul(pssc[:], wt_r[:, :, dk, b],
                                         rawT[:, dk, b, sl],
                                         start=(dk == 0), stop=(dk == 3))
                    s4 = attw.tile([4, 512], F32, tag="s4", name="s4", bufs=3)
                    if ei % 2 == 0:
                        nc.scalar.copy(s4[:], pssc[:])
                    else:
                        nc.vector.tensor_copy(s4[:], pssc[:])
                    nc.sync.dma_start(scores_sb[b * 4:(b + 1) * 4, sl], s4[:])
                    ei += 1
            rmax = attw.tile([16, 1], F32, tag="rmax", name="rmax")
            nc.vector.tensor_reduce(out=rmax[:], in_=scores_sb[:],
                                    axis=mybir.AxisListType.X, op=ALU.max)
            nmax = attw.tile([16, 1], F32, tag="nmax", name="nmax")
            nc.vector.tensor_scalar_mul(nmax[:], rmax[:], -1.0)
            e_sb = attw.tile([16, S], F32, tag="e_sb", name="e_sb", bufs=1)
            nc.scalar.activation(e_sb[:], scores_sb[:], AF.Exp, bias=nmax[:],
                                 scale=1.0)
            zs = attw.tile([16, 1], F32, tag="zs", name="zs")
            nc.vector.tensor_reduce(out=zs[:], in_=e_sb[:],
                                    axis=mybir.AxisListType.X, op=ALU.add)
            rz = attw.tile([16, 1], F32, tag="rz", name="rz")
            nc.vector.reciprocal(rz[:], zs[:])
            attn = att.tile([16, S], FP16, tag="attn", name="attn")
            nc.vector.tensor_scalar_mul(attn[:], e_sb[:], rz[:])

            # rst[e, s] = sum_dk wq[dk, h, e] rawT[dk, b, s] + v[e] attn[s]
            ei = 0
            for b in range(BC):
                rstT = attw.tile([128, NH, S], FP16, tag="rstT", name="rstT")
                for h in range(NH):
                    row = b * 4 + h
                    attn1 = attw.tile([1, S], FP16, tag="attn1", name="attn1",
                                      bufs=3)
                    nc.sync.dma_start(attn1[:], attn[row:row + 1, :])
                    for half in range(2):
                        sl = slice(half * 512, (half + 1) * 512)
                        psr = r_ps.tile([128, 512], F32, tag="rst", name="rst")
                        for dk in range(4):
                            nc.tensor.matmul(psr[:], wq_r[:, dk, h, :],
                                             rawT[:, dk, b, sl],
                                             start=(dk == 0), stop=False)
                        nc.tensor.matmul(psr[:], v1[:, b, h, :],
                                         attn1[:, sl],
                                         start=False, stop=True)
                        if ei % 2 == 0:
                            nc.scalar.copy(rstT[:, h, sl], psr[:])
                        else:
                            nc.vector.tensor_copy(rstT[:, h, sl], psr[:])
                        ei += 1
                for tch in range(8):
                    obuf = attw.tile([128, NH, DH], F32, tag="obuf", name="obuf",
                                     bufs=3)
                    for h in range(NH):
                        ps_t = t_ps.tile([128, DH], FP16, tag="ps_t", name="ps_t")
                        nc.tensor.transpose(
                            ps_t[:], rstT[:, h, tch * 128:(tch + 1) * 128],
                            identh[:])
                        if h % 2 == 0:
                            nc.scalar.copy(obuf[:, h, :], ps_t[:])
                        else:
                            nc.vector.tensor_copy(obuf[:, h, :], ps_t[:])
                    nc.sync.dma_start(
                        out_d[b, tch * 128:(tch + 1) * 128, :],
                        obuf[:].rearrange("p h e -> p (h e)"))

        persist.release()

    nc.compile()
    return nc


# ======================= host-side prep =======================

_GS = np.concatenate([np.full(2 * H, 0.5, np.float32),
                      np.full(H, 1.0, np.float32),
                      np.full(H, 0.5, np.float32)])  # i,f,o tanh-trick scaling


def _prep_shared(inputs):
    sh = {}
    for nm, pre in [("rf", "raw_f"), ("rb", "raw_b"), ("sf", "sum_f"), ("sb", "sum_b")]:
        wihm = np.asarray(inputs[pre + "_Wih"], np.float32)   # [1024, 300]
        bb = np.asarray(inputs[pre + "_b"], np.float32)
        whhm = np.asarray(inputs[pre + "_Whh"], np.float32)   # [1024, 256]
        aug = np.zeros((384, 4 * H), np.float32)
        aug[:D_IN] = wihm.T
        aug[D_IN] = bb
        aug *= _GS[None, :]
        sh[f"wih_{nm}"] = np.ascontiguousarray(
            aug.astype(np.float16).reshape(3, 128, 8, 128))
        whhT = (whhm.T * _GS[None, :] * 0.5).astype(np.float16)
        sh[f"whh_{nm}"] = np.ascontiguousarray(whhT.reshape(2, 128, 8, 128))
    wq = np.asarray(inputs["Wq"], np.float32) * 0.5          # [NH, 512, 128]
    wq4 = wq.reshape(NH, 4, 128, DH)
    sh["wqst"] = np.ascontiguousarray(
        np.transpose(wq4, (1, 2, 0, 3)).astype(np.float16))
    sh["wqtt"] = np.ascontiguousarray(
        np.transpose(wq4, (3, 0, 1, 2)).astype(np.float16))
    wk4 = np.asarray(inputs["Wk"], np.float32).reshape(NH, 4, 128, DH)
    wv4 = np.asarray(inputs["Wv"], np.float32).reshape(NH, 4, 128, DH)
    sh["wkv"] = np.ascontiguousarray(
        np.stack([np.transpose(wk4, (2, 0, 1, 3)),
                  np.transpose(wv4, (2, 0, 1, 3))], axis=1).astype(np.float16))
    return sh


def _prep_core_inputs(c, inputs, shared):
    rows = slice(c * BC, (c + 1) * BC)
    m = dict(shared)
    xr = np.zeros((384, BC, XR), np.float32)
    xr[:D_IN, :, PAD:PAD + S] = np.transpose(np.asarray(inputs["in_raw"][rows],
                                                        np.float32), (2, 0, 1))
    xr[D_IN] = 1.0
    m["xr"] = np.ascontiguousarray(
        xr.astype(np.float16).reshape(3, 128, BC, XR))
    xsv = np.zeros((384, BC, XS), np.float32)
    xsv[:D_IN, :, PAD:PAD + SS] = np.transpose(np.asarray(inputs["in_sum"][rows],
                                                          np.float32), (2, 0, 1))
    xsv[D_IN] = 1.0
    m["xs"] = np.ascontiguousarray(
        xsv.astype(np.float16).reshape(3, 128, BC, XS))
    lens = np.asarray(inputs["len_sum"][rows])
    mask = (np.arange(SS)[None, :] < lens[:, None]).astype(np.float32)
    m["maskdiv"] = np.ascontiguousarray(
        (mask * 0.5 / np.maximum(lens, 1).astype(np.float32)[:, None])
        .astype(np.float16))
    return m


_NC_CACHE = {}


def get_nc():
    if "nc" not in _NC_CACHE:
        _NC_CACHE["nc"] = build_nc()
    return _NC_CACHE["nc"]


def kernel(**inputs) -> np.ndarray:
    nc = get_nc()
    shared = _prep_shared(inputs)
    in_maps = [_prep_core_inputs(c, inputs, shared) for c in range(NCORES)]
    trace = bool(int(os.environ.get("K_TRACE", "0")))
    res = bass_utils.run_bass_kernel_spmd(
        nc, in_maps, core_ids=list(range(NCORES)), trace=trace)
    if trace and res.exec_time_ns is not None:
        print(f"HW exec time: {res.exec_time_ns} ns")
        kernel.last_exec_ns = res.exec_time_ns
    kernel.last_results = res
    out = np.concatenate([res.results[c]["out"] for c in range(NCORES)], axis=0)
    return out


# revision 16
# speedup vs baseline: 1.1943x; 1.1943x over previous
"""Trainium2 Bass kernel for nn_BiLSTM_centric_layer — segmented-recurrence design.

Key ideas vs the naive data-parallel kernel:

1. The LSTM recurrence is LATENCY-bound on TRN2: each chain's loop-carried
   dependency (matmul -> tanh(gates) -> cell update -> tanh(C) -> h) costs
   ~2-3us of semaphore/engine-init latency per step, and plain BiLSTM only
   exposes 2 independent chains (fwd/bwd). We exploit exponential LSTM state
   forgetting (prod of sigmoid(f) decays ~e^-0.45/step) to split the sequence
   into P segments, each warmed up W steps from zero state (numerically
   validated: W=16 -> ~3e-4 state error, decaying further downstream).
   This gives 2P independent chains.

2. K=4 segments advance in lockstep inside single fat instructions (one
   tanh over all of them), amortizing the ~370ns fixed cost of each
   Activation-engine op; R=5 such groups rotate to hide the dependency
   latency.

3. Input-gate terms x@Wih are accumulated straight into the gates PSUM tile
   by the PE (3 extra tiny matmuls per gate chunk) — no xg precompute pass,
   no DRAM round trip, no vector-engine add.

4. h' = 2h = (tanh_o + 1) * tanh(C) is ONE fused DVE op
   (scalar_tensor_tensor); the 0.5 factors are folded into Whh, Wq and the
   mean-pool mask on the host. h' is written fp16 directly into the SBUF
   history tensor which the next step's matmul reads in place.

5. Attention scores are computed directly from the history via
   w_tilde = (0.5 Wq) @ k (rank-1), so q is only materialized inside the
   fused q + attn (x) v PSUM accumulation.

Sharding: data-parallel over batch (4 rows/core x 8 cores), weights
replicated. Everything hardcoded for B=32, S_RAW=1024, S_SUM=128, D_IN=300,
H=256, NH=4.
"""
import os
import sys

sys.path.insert(0, "/opt/trn_rl_repo")

import numpy as np
import ml_dtypes

import concourse.bacc as bacc
import concourse.bass as bass
import concourse.mybir as mybir
import concourse.tile as tile
from concourse import bass_utils
from concourse.masks import make_identity

F32 = mybir.dt.float32
F32R = mybir.dt.float32r
FP16 = mybir.dt.bfloat16  # bf16: fast PE path
AF = mybir.ActivationFunctionType
ALU = mybir.AluOpType

B, S, SS, D_IN, H, NH = 32, 1024, 128, 300, 256, 4
DH = 128
BC = 4             # batch rows per core
NCORES = 8
PAD = 32           # zero padding (timesteps) on both ends of x
XR = S + 2 * PAD   # padded raw length
XS = SS + 2 * PAD  # padded sum length
W = int(os.environ.get("K_W", "16"))            # warmup steps
assert 1 <= W <= PAD

# rotation groups: (base, K segments, seg_len) covering [base, base+K*L);
# fwd seg j = [base+j*L, ...); bwd mirrored. Fat K amortizes the ~36ns
# per-matmul weight-load cost; >=2 raw groups hide the chain latency.
RAW_GROUPS = [(0, 16, 32), (512, 16, 32)]
assert sum(Kg * L for _, Kg, L in RAW_GROUPS) == S
SUM_GROUPS = [(0, 16, 8)]
assert sum(Kg * L for _, Kg, L in SUM_GROUPS) == SS


class Group:
    def __init__(self, gid, base, Kg, L, dirsets, Sg, is_sum):
        self.id, self.base, self.K, self.L = gid, base, Kg, L
        self.dirsets, self.Sg, self.is_sum = dirsets, Sg, is_sum
        self.rounds = W + L


def _tap(t, off, dims):
    full = t[:]
    return bass.AP(tensor=full.tensor, offset=full.offset + off,
                   ap=[list(full.ap[0])] + [list(d) for d in dims])


def build_nc():
    nc = bacc.Bacc("TRN2", target_bir_lowering=False, debug=False)

    # ---- DRAM I/O ----
    xr_d = nc.dram_tensor("xr", [3, 128, BC, XR], FP16, kind="ExternalInput")
    xs_d = nc.dram_tensor("xs", [3, 128, BC, XS], FP16, kind="ExternalInput")
    wih_d, whh_d = {}, {}
    for nm in ["rf", "rb", "sf", "sb"]:
        wih_d[nm] = nc.dram_tensor(f"wih_{nm}", [3, 128, 8, 128], FP16,
                                   kind="ExternalInput")
        whh_d[nm] = nc.dram_tensor(f"whh_{nm}", [2, 128, 8, 128], FP16,
                                   kind="ExternalInput")
    wqst_d = nc.dram_tensor("wqst", [4, 128, NH, DH], FP16, kind="ExternalInput")
    wqtt_d = nc.dram_tensor("wqtt", [128, NH, 4, DH], FP16, kind="ExternalInput")
    wkv_d = nc.dram_tensor("wkv", [128, 2, NH, 4, DH], FP16, kind="ExternalInput")
    maskdiv_d = nc.dram_tensor("maskdiv", [BC, SS], FP16, kind="ExternalInput")
    out_d = nc.dram_tensor("out", [BC, S, NH * DH], F32, kind="ExternalOutput")

    with tile.TileContext(nc) as tc:
        persist = tc.alloc_tile_pool(name="persist", bufs=1)
        rec = tc.alloc_tile_pool(name="rec", bufs=1)        # released before E
        work = tc.alloc_tile_pool(name="work", bufs=2)
        gps = tc.alloc_tile_pool(name="gps", bufs=1, space="PSUM")

        ident = persist.tile([128, 128], F32, tag="ident", name="ident")
        make_identity(nc, ident[:])
        identh = persist.tile([128, 128], FP16, tag="identh", name="identh")
        nc.vector.tensor_copy(identh[:], ident[:])

        # ---- stage inputs in SBUF ----
        xr_sb = rec.tile([128, 3, BC, XR], FP16, tag="xr", name="xr_sb")
        nc.sync.dma_start(xr_sb[:], xr_d[:].rearrange("kc p b t -> p kc b t"))
        xs_sb = rec.tile([128, 3, BC, XS], FP16, tag="xs", name="xs_sb")
        nc.sync.dma_start(xs_sb[:], xs_d[:].rearrange("kc p b t -> p kc b t"))
        wih, whh = {}, {}
        for nm in ["rf", "rb", "sf", "sb"]:
            wih[nm] = rec.tile([128, 3, 8, 128], FP16, tag=f"wih{nm}", name=f"wih{nm}")
            nc.sync.dma_start(wih[nm][:], wih_d[nm][:].rearrange("kc p mc c -> p kc mc c"))
            whh[nm] = rec.tile([128, 2, 8, 128], FP16, tag=f"whh{nm}", name=f"whh{nm}")
            nc.sync.dma_start(whh[nm][:], whh_d[nm][:].rearrange("kc p mc c -> p kc mc c"))

        # history tensors (fp16, hold h' = 2h)
        rawT = persist.tile([128, 4, BC, S], FP16, tag="rawT", name="rawT")
        sumT = persist.tile([128, 4, BC, SS], FP16, tag="sumT", name="sumT")

        # ---- group state ----
        groups = []
        for gi, (base, Kg, L) in enumerate(RAW_GROUPS):
            groups.append(Group(gi, base, Kg, L, ("rf", "rb"), S, False))
        for gi, (base, Kg, L) in enumerate(SUM_GROUPS):
            groups.append(Group(len(RAW_GROUPS) + gi, base, Kg, L,
                                ("sf", "sb"), SS, True))

        C, hs = {}, {}
        for g in groups:
            C[g.id] = rec.tile([128, 2, 2, g.K, BC], F32, tag=f"C{g.id}",
                               name=f"C{g.id}")
            nc.vector.memset(C[g.id][:], 0.0)
            hs[g.id] = []
            for par in range(2):
                t = rec.tile([128, 2, 2, g.K, BC], FP16, tag=f"hs{g.id}_{par}",
                             name=f"hs{g.id}_{par}")
                hs[g.id].append(t)

        def xcol0(g, rr, d):
            # x column (into padded buffer) for segment 0 at round rr
            if d == 0:
                return PAD + g.base + (rr - W)
            return PAD + (g.Sg - g.base) + (W - 1) - rr

        def hist_t0(g, rstep, d):
            # history t for segment 0 at real step rstep
            if d == 0:
                return g.base + rstep
            return g.Sg - 1 - g.base - rstep

        def emit_round(g, rr):
            xsb = xs_sb if g.is_sum else xr_sb
            XL = XS if g.is_sum else XR
            hist = sumT if g.is_sum else rawT
            Sg, L, Kg = g.Sg, g.L, g.K
            ps = gps.tile([128, 2, 8, Kg, BC], F32, tag=f"ps{g.id}", name=f"ps{g.id}")
            for d in (0, 1):
                st = L if d == 0 else -L
                c0 = xcol0(g, rr, d)
                wi, wh = wih[g.dirsets[d]], whh[g.dirsets[d]]
                for mc in range(8):
                    o = ps[:, d, mc, :, :]
                    for kc in range(3):
                        mv = _tap(xsb, kc * (BC * XL) + c0, [[st, Kg], [XL, BC]])
                        nc.tensor.matmul(o, wi[:, kc, mc, :], mv,
                                         start=(kc == 0),
                                         stop=(kc == 2 and rr == 0))
                    if rr > 0:
                        for kc in range(2):
                            if rr - 1 < W:
                                hm = hs[g.id][(rr - 1) % 2][:, d, kc, :, :]
                            else:
                                t0p = hist_t0(g, rr - 1 - W, d)
                                hm = _tap(hist, (2 * d + kc) * BC * Sg + t0p,
                                          [[st, Kg], [Sg, BC]])
                            nc.tensor.matmul(o, wh[:, kc, mc, :], hm,
                                             start=False, stop=(kc == 1))
            th = work.tile([128, 2, 8, Kg, BC], FP16, tag=f"th{g.id}", name=f"th{g.id}")
            nc.scalar.activation(th[:], ps[:], AF.Tanh)
            # state D = 2C:  D' = (tf+1)*0.5*D + (ti+1)*tg ; tc = tanh(0.5 D')
            ths = {blk: th[:, :, 2 * blk:2 * blk + 2, :, :]
                   .rearrange("p d m k b -> p d m (k b)") for blk in range(4)}
            Dap = C[g.id][:].rearrange("p d m k b -> p d m (k b)")
            u = work.tile([128, 2, 2, Kg * BC], F32, tag=f"u{g.id}", name=f"u{g.id}")
            nc.vector.scalar_tensor_tensor(out=u[:], in0=ths[0], scalar=1.0,
                                           in1=ths[2], op0=ALU.add, op1=ALU.mult)
            v = work.tile([128, 2, 2, Kg * BC], F32, tag=f"v{g.id}", name=f"v{g.id}")
            nc.vector.scalar_tensor_tensor(out=v[:], in0=ths[1], scalar=1.0,
                                           in1=Dap, op0=ALU.add, op1=ALU.mult)
            nc.vector.scalar_tensor_tensor(out=Dap, in0=v[:], scalar=0.5,
                                           in1=u[:], op0=ALU.mult, op1=ALU.add)
            tcl = work.tile([128, 2, 2, Kg, BC], FP16, tag=f"tc{g.id}", name=f"tc{g.id}")
            nc.scalar.activation(tcl[:], C[g.id][:], AF.Tanh, scale=0.5)
            for d in (0, 1):
                eng = nc.vector
                for kc in range(2):
                    if rr < W:
                        dst = hs[g.id][rr % 2][:, d, kc, :, :]
                    else:
                        st = L if d == 0 else -L
                        t0 = hist_t0(g, rr - W, d)
                        dst = _tap(hist, (2 * d + kc) * BC * Sg + t0,
                                   [[st, Kg], [Sg, BC]])
                    eng.scalar_tensor_tensor(
                        out=dst, in0=th[:, d, 6 + kc, :, :], scalar=1.0,
                        in1=tcl[:, d, kc, :, :], op0=ALU.add, op1=ALU.mult)
            if rr == W - 1 and g.base == 0:
                # segment 0 (fwd [0,L), bwd [Sg-L,Sg)) starts from the true
                # zero state: discard its garbage warmup state.
                nc.vector.memset(C[g.id][:, :, :, 0, :], 0.0)
                nc.vector.memset(hs[g.id][(W - 1) % 2][:, :, :, 0, :], 0.0)

        # phase D tiles that later phases need
        kT_r = persist.tile([128, NH, BC], FP16, tag="kT_r", name="kT_r")
        v1 = persist.tile([1, BC, NH, DH], FP16, tag="v1", name="v1")
        wt_r = persist.tile([128, NH, 4, BC], FP16, tag="wt_r", name="wt_r")

        def emit_phase_d():
            with tc.tile_pool(name="dpool", bufs=1) as pl, \
                 tc.tile_pool(name="d_ps", bufs=1, space="PSUM") as dps:
                msk = pl.tile([128, 4, BC, SS], FP16, tag="msk", name="msk")
                srcap = bass.AP(tensor=maskdiv_d, offset=0,
                                ap=[[0, 128], [SS, BC], [1, SS]])
                for dk in range(4):
                    nc.sync.dma_start(msk[:, dk, :, :], srcap)
                masked = pl.tile([128, 4, BC, SS], FP16, tag="masked", name="masked")
                nc.vector.tensor_tensor(out=masked[:], in0=sumT[:], in1=msk[:],
                                        op=ALU.mult)
                sv = pl.tile([128, 4, BC], F32, tag="sv", name="sv")
                nc.vector.tensor_reduce(out=sv[:], in_=masked[:],
                                        axis=mybir.AxisListType.X, op=ALU.add)
                sv_h = pl.tile([128, 4, BC], FP16, tag="sv_h", name="sv_h")
                nc.vector.tensor_copy(sv_h[:], sv[:])

                wkv = pl.tile([128, 2, NH, 4, DH], FP16, tag="wkv", name="wkv")
                nc.sync.dma_start(wkv[:], wkv_d[:])
                ps_kv = dps.tile([128, NH, 2, BC], F32, tag="dps", name="ps_kv")
                for h in range(NH):
                    for ih in range(2):
                        for dk in range(4):
                            nc.tensor.matmul(ps_kv[:, h, ih, :],
                                             wkv[:, ih, h, dk, :],
                                             sv_h[:, dk, :],
                                             start=(dk == 0), stop=(dk == 3))
                nc.vector.tensor_copy(kT_r[:], ps_kv[:, :, 0, :])
                v_sb = pl.tile([128, NH, BC], FP16, tag="v_sb", name="v_sb")
                nc.scalar.copy(v_sb[:], ps_kv[:, :, 1, :])
                ps_vt = dps.tile([BC, NH, DH], FP16, tag="dps", name="ps_vt")
                for h in range(NH):
                    nc.tensor.transpose(ps_vt[:, h, :], v_sb[:, h, :], identh[:])
                v4 = pl.tile([BC, NH, DH], FP16, tag="v4", name="v4")
                nc.vector.tensor_copy(v4[:], ps_vt[:])
                for b in range(BC):
                    nc.sync.dma_start(v1[:, b, :, :], v4[b:b + 1, :, :])

                # w_tilde[dmod, h, dk, b] = sum_e (0.5 Wq)[h][dk*128+dmod, e] k[e, h, b]
                wqtt = pl.tile([128, NH, 4, DH], FP16, tag="wqtt", name="wqtt")
                nc.sync.dma_start(wqtt[:], wqtt_d[:])
                ps_wt = dps.tile([128, NH, 4, BC], F32, tag="dps", name="ps_wt")
                for h in range(NH):
                    for dk in range(4):
                        nc.tensor.matmul(ps_wt[:, h, dk, :], wqtt[:, h, dk, :],
                                         kT_r[:, h, :], start=True, stop=True)
                nc.vector.tensor_copy(wt_r[:], ps_wt[:])

        # ================= recurrence rotation =================
        max_rounds = max(g.rounds for g in groups)
        sum_last = max(g.rounds for g in groups if g.is_sum) - 1
        for rr in range(max_rounds):
            for g in groups:
                if rr < g.rounds:
                    emit_round(g, rr)
            if rr == sum_last:
                emit_phase_d()

        gps.release()
        work.release()
        rec.release()

        # ================= phase E: scores, softmax, rst =================
        with tc.tile_pool(name="att", bufs=1) as att, \
             tc.tile_pool(name="attw", bufs=2) as attw, \
             tc.tile_pool(name="sc_ps", bufs=3, space="PSUM") as sc_ps, \
             tc.tile_pool(name="r_ps", bufs=3, space="PSUM") as r_ps, \
             tc.tile_pool(name="t_ps", bufs=2, space="PSUM") as t_ps:
            wq_r = att.tile([128, 4, NH, DH], FP16, tag="wq_r", name="wq_r")
            nc.sync.dma_start(wq_r[:], wqst_d[:].rearrange("dk p h e -> p dk h e"))

            # scores[b*4+h, s] = sum_d rawT[d, b, s] * wt[d, h, b]
            scores_sb = att.tile([16, S], F32, tag="scores", name="scores")
            ei = 0
            for b in range(BC):
                for half in range(2):
                    sl = slice(half * 512, (half + 1) * 512)
                    pssc = sc_ps.tile([4, 512], F32, tag="sc", name="sc")
                    for dk in range(4):
                        nc.tensor.matmul(pssc[:], wt_r[:, :, dk, b],
                                         rawT[:, dk, b, sl],
                                         start=(dk == 0), stop=(dk == 3))
                    s4 = attw.tile([4, 512], F32, tag="s4", name="s4", bufs=3)
                    if ei % 2 == 0:
                        nc.scalar.copy(s4[:], pssc[:])
                    else:
                        nc.vector.tensor_copy(s4[:], pssc[:])
                    nc.sync.dma_start(scores_sb[b * 4:(b + 1) * 4, sl], s4[:])
                    ei += 1
            rmax = attw.tile([16, 1], F32, tag="rmax", name="rmax")
            nc.vector.tensor_reduce(out=rmax[:], in_=scores_sb[:],
                                    axis=mybir.AxisListType.X, op=ALU.max)
            nmax = attw.tile([16, 1], F32, tag="nmax", name="nmax")
            nc.vector.tensor_scalar_mul(nmax[:], rmax[:], -1.0)
            e_sb = attw.tile([16, S], F32, tag="e_sb", name="e_sb", bufs=1)
            nc.scalar.activation(e_sb[:], scores_sb[:], AF.Exp, bias=nmax[:],
                                 scale=1.0)
            zs = attw.tile([16, 1], F32, tag="zs", name="zs")
            nc.vector.tensor_reduce(out=zs[:], in_=e_sb[:],
                                    axis=mybir.AxisListType.X, op=ALU.add)
            rz = attw.tile([16, 1], F32, tag="rz", name="rz")
            nc.vector.reciprocal(rz[:], zs[:])
            attn = att.tile([16, S], FP16, tag="attn", name="attn")
            nc.vector.tensor_scalar_mul(attn[:], e_sb[:], rz[:])

            # rst[e, s] = sum_dk wq[dk, h, e] rawT[dk, b, s] + v[e] attn[s]
            ei = 0
            for b in range(BC):
                rstT = attw.tile([128, NH, S], FP16, tag="rstT", name="rstT")
                for h in range(NH):
                    row = b * 4 + h
                    attn1 = attw.tile([1, S], FP16, tag="attn1", name="attn1",
                                      bufs=3)
                    nc.sync.dma_start(attn1[:], attn[row:row + 1, :])
                    for half in range(2):
                        sl = slice(half * 512, (half + 1) * 512)
                        psr = r_ps.tile([128, 512], F32, tag="rst", name="rst")
                        for dk in range(4):
                            nc.tensor.matmul(psr[:], wq_r[:, dk, h, :],
                                             rawT[:, dk, b, sl],
                                             start=(dk == 0), stop=False)
                        nc.tensor.matmul(psr[:], v1[:, b, h, :],
                                         attn1[:, sl],
                                         start=False, stop=True)
                        if ei % 2 == 0:
                            nc.scalar.copy(rstT[:, h, sl], psr[:])
                        else:
                            nc.vector.tensor_copy(rstT[:, h, sl], psr[:])
                        ei += 1
                for tch in range(8):
                    obuf = attw.tile([128, NH, DH], F32, tag="obuf", name="obuf",
                                     bufs=3)
                    for h in range(NH):
                        ps_t = t_ps.tile([128, DH], FP16, tag="ps_t", name="ps_t")
                        nc.tensor.transpose(
                            ps_t[:], rstT[:, h, tch * 128:(tch + 1) * 128],
                            identh[:])
                        if h % 2 == 0:
                            nc.scalar.copy(obuf[:, h, :], ps_t[:])
                        else:
                            nc.vector.tensor_copy(obuf[:, h, :], ps_t[:])
                    nc.sync.dma_start(
                        out_d[b, tch * 128:(tch + 1) * 128, :],
                        obuf[:].rearrange("p h e -> p (h e)"))

        persist.release()

    nc.compile()
    return nc


# ======================= host-side prep =======================

_GS = np.concatenate([np.full(2 * H, 0.5, np.float32),
                      np.full(H, 1.0, np.float32),
                      np.full(H, 0.5, np.float32)])  # i,f,o tanh-trick scaling


def _prep_shared(inputs):
    sh = {}
    for nm, pre in [("rf", "raw_f"), ("rb", "raw_b"), ("sf", "sum_f"), ("sb", "sum_b")]:
        wihm = np.asarray(inputs[pre + "_Wih"], np.float32)   # [1024, 300]
        bb = np.asarray(inputs[pre + "_b"], np.float32)
        whhm = np.asarray(inputs[pre + "_Whh"], np.float32)   # [1024, 256]
        aug = np.zeros((384, 4 * H), np.float32)
        aug[:D_IN] = wihm.T
        aug[D_IN] = bb
        aug *= _GS[None, :]
        sh[f"wih_{nm}"] = np.ascontiguousarray(
            aug.astype(ml_dtypes.bfloat16).reshape(3, 128, 8, 128))
        whhT = (whhm.T * _GS[None, :] * 0.5).astype(ml_dtypes.bfloat16)
        sh[f"whh_{nm}"] = np.ascontiguousarray(whhT.reshape(2, 128, 8, 128))
    wq = np.asarray(inputs["Wq"], np.float32) * 0.5          # [NH, 512, 128]
    wq4 = wq.reshape(NH, 4, 128, DH)
    sh["wqst"] = np.ascontiguousarray(
        np.transpose(wq4, (1, 2, 0, 3)).astype(ml_dtypes.bfloat16))
    sh["wqtt"] = np.ascontiguousarray(
        np.transpose(wq4, (3, 0, 1, 2)).astype(ml_dtypes.bfloat16))
    wk4 = np.asarray(inputs["Wk"], np.float32).reshape(NH, 4, 128, DH)
    wv4 = np.asarray(inputs["Wv"], np.float32).reshape(NH, 4, 128, DH)
    sh["wkv"] = np.ascontiguousarray(
        np.stack([np.transpose(wk4, (2, 0, 1, 3)),
                  np.transpose(wv4, (2, 0, 1, 3))], axis=1).astype(ml_dtypes.bfloat16))
    return sh


def _prep_core_inputs(c, inputs, shared):
    rows = slice(c * BC, (c + 1) * BC)
    m = dict(shared)
    xr = np.zeros((384, BC, XR), np.float32)
    xr[:D_IN, :, PAD:PAD + S] = np.transpose(np.asarray(inputs["in_raw"][rows],
                                                        np.float32), (2, 0, 1))
    xr[D_IN] = 1.0
    m["xr"] = np.ascontiguousarray(
        xr.astype(ml_dtypes.bfloat16).reshape(3, 128, BC, XR))
    xsv = np.zeros((384, BC, XS), np.float32)
    xsv[:D_IN, :, PAD:PAD + SS] = np.transpose(np.asarray(inputs["in_sum"][rows],
                                                          np.float32), (2, 0, 1))
    xsv[D_IN] = 1.0
    m["xs"] = np.ascontiguousarray(
        xsv.astype(ml_dtypes.bfloat16).reshape(3, 128, BC, XS))
    lens = np.asarray(inputs["len_sum"][rows])
    mask = (np.arange(SS)[None, :] < lens[:, None]).astype(np.float32)
    m["maskdiv"] = np.ascontiguousarray(
        (mask * 0.5 / np.maximum(lens, 1).astype(np.float32)[:, None])
        .astype(ml_dtypes.bfloat16))
    return m


_NC_CACHE = {}


def get_nc():
    if "nc" not in _NC_CACHE:
        _NC_CACHE["nc"] = build_nc()
    return _NC_CACHE["nc"]


def kernel(**inputs) -> np.ndarray:
    nc = get_nc()
    shared = _prep_shared(inputs)
    in_maps = [_prep_core_inputs(c, inputs, shared) for c in range(NCORES)]
    trace = bool(int(os.environ.get("K_TRACE", "0")))
    res = bass_utils.run_bass_kernel_spmd(
        nc, in_maps, core_ids=list(range(NCORES)), trace=trace)
    if trace and res.exec_time_ns is not None:
        print(f"HW exec time: {res.exec_time_ns} ns")
        kernel.last_exec_ns = res.exec_time_ns
    kernel.last_results = res
    out = np.concatenate([res.results[c]["out"] for c in range(NCORES)], axis=0)
    return out


# revision 17
# speedup vs baseline: 1.5367x; 1.2866x over previous
"""Trainium2 Bass kernel for nn_BiLSTM_centric_layer — segmented-recurrence design.

Key ideas vs the naive data-parallel kernel:

1. The LSTM recurrence is LATENCY-bound on TRN2: each chain's loop-carried
   dependency (matmul -> tanh(gates) -> cell update -> tanh(C) -> h) costs
   ~2-3us of semaphore/engine-init latency per step, and plain BiLSTM only
   exposes 2 independent chains (fwd/bwd). We exploit exponential LSTM state
   forgetting (prod of sigmoid(f) decays ~e^-0.45/step) to split the sequence
   into P segments, each warmed up W steps from zero state (numerically
   validated: W=16 -> ~3e-4 state error, decaying further downstream).
   This gives 2P independent chains.

2. K=4 segments advance in lockstep inside single fat instructions (one
   tanh over all of them), amortizing the ~370ns fixed cost of each
   Activation-engine op; R=5 such groups rotate to hide the dependency
   latency.

3. Input-gate terms x@Wih are accumulated straight into the gates PSUM tile
   by the PE (3 extra tiny matmuls per gate chunk) — no xg precompute pass,
   no DRAM round trip, no vector-engine add.

4. h' = 2h = (tanh_o + 1) * tanh(C) is ONE fused DVE op
   (scalar_tensor_tensor); the 0.5 factors are folded into Whh, Wq and the
   mean-pool mask on the host. h' is written fp16 directly into the SBUF
   history tensor which the next step's matmul reads in place.

5. Attention scores are computed directly from the history via
   w_tilde = (0.5 Wq) @ k (rank-1), so q is only materialized inside the
   fused q + attn (x) v PSUM accumulation.

Sharding: data-parallel over batch (4 rows/core x 8 cores), weights
replicated. Everything hardcoded for B=32, S_RAW=1024, S_SUM=128, D_IN=300,
H=256, NH=4.
"""
import os
import sys

sys.path.insert(0, "/opt/trn_rl_repo")

import numpy as np
import ml_dtypes

import concourse.bacc as bacc
import concourse.bass as bass
import concourse.mybir as mybir
import concourse.tile as tile
from concourse import bass_utils
from concourse.masks import make_identity

F32 = mybir.dt.float32
F32R = mybir.dt.float32r
FP16 = mybir.dt.bfloat16  # bf16: fast PE path
AF = mybir.ActivationFunctionType
ALU = mybir.AluOpType

B, S, SS, D_IN, H, NH = 32, 1024, 128, 300, 256, 4
DH = 128
BC = 4             # batch rows per core
NCORES = 8
PAD = 32           # zero padding (timesteps) on both ends of x
XR = S + 2 * PAD   # padded raw length
XS = SS + 2 * PAD  # padded sum length
W = int(os.environ.get("K_W", "12"))            # warmup steps
assert 1 <= W <= PAD

# rotation groups: (base, K segments, seg_len) covering [base, base+K*L);
# fwd seg j = [base+j*L, ...); bwd mirrored. Fat K amortizes the ~36ns
# per-matmul weight-load cost; >=2 raw groups hide the chain latency.
RAW_GROUPS = [(0, 16, 32), (512, 16, 32)]
assert sum(Kg * L for _, Kg, L in RAW_GROUPS) == S
SUM_GROUPS = [(0, 8, 16)]
assert sum(Kg * L for _, Kg, L in SUM_GROUPS) == SS
XWIN = 4           # x-projection batch window (rounds) for raw groups


class Group:
    def __init__(self, gid, base, Kg, L, dirsets, Sg, is_sum):
        self.id, self.base, self.K, self.L = gid, base, Kg, L
        self.dirsets, self.Sg, self.is_sum = dirsets, Sg, is_sum
        self.rounds = W + L
        self.xwin = None if is_sum else XWIN


def _tap(t, off, dims):
    full = t[:]
    return bass.AP(tensor=full.tensor, offset=full.offset + off,
                   ap=[list(full.ap[0])] + [list(d) for d in dims])


def build_nc():
    nc = bacc.Bacc("TRN2", target_bir_lowering=False, debug=False)

    # ---- DRAM I/O ----
    xr_d = nc.dram_tensor("xr", [3, 128, BC, XR], FP16, kind="ExternalInput")
    xs_d = nc.dram_tensor("xs", [3, 128, BC, XS], FP16, kind="ExternalInput")
    wih_d, whh_d = {}, {}
    for nm in ["rf", "rb", "sf", "sb"]:
        wih_d[nm] = nc.dram_tensor(f"wih_{nm}", [3, 128, 8, 128], FP16,
                                   kind="ExternalInput")
        whh_d[nm] = nc.dram_tensor(f"whh_{nm}", [2, 128, 8, 128], FP16,
                                   kind="ExternalInput")
    wqst_d = nc.dram_tensor("wqst", [4, 128, NH, DH], FP16, kind="ExternalInput")
    wqtt_d = nc.dram_tensor("wqtt", [128, NH, 4, DH], FP16, kind="ExternalInput")
    wkv_d = nc.dram_tensor("wkv", [128, 2, NH, 4, DH], FP16, kind="ExternalInput")
    maskdiv_d = nc.dram_tensor("maskdiv", [BC, SS], FP16, kind="ExternalInput")
    out_d = nc.dram_tensor("out", [BC, S, NH * DH], F32, kind="ExternalOutput")

    with tile.TileContext(nc) as tc:
        persist = tc.alloc_tile_pool(name="persist", bufs=1)
        rec = tc.alloc_tile_pool(name="rec", bufs=1)        # released before E
        work = tc.alloc_tile_pool(name="work", bufs=2)
        gps = tc.alloc_tile_pool(name="gps", bufs=1, space="PSUM")
        xbp = tc.alloc_tile_pool(name="xbp", bufs=2, space="PSUM")

        ident = persist.tile([128, 128], F32, tag="ident", name="ident")
        make_identity(nc, ident[:])
        identh = persist.tile([128, 128], FP16, tag="identh", name="identh")
        nc.vector.tensor_copy(identh[:], ident[:])

        # ---- stage inputs in SBUF ----
        xr_sb = rec.tile([128, 3, BC, XR], FP16, tag="xr", name="xr_sb")
        nc.sync.dma_start(xr_sb[:], xr_d[:].rearrange("kc p b t -> p kc b t"))
        xs_sb = rec.tile([128, 3, BC, XS], FP16, tag="xs", name="xs_sb")
        nc.sync.dma_start(xs_sb[:], xs_d[:].rearrange("kc p b t -> p kc b t"))
        wih, whh = {}, {}
        for nm in ["rf", "rb", "sf", "sb"]:
            wih[nm] = rec.tile([128, 3, 8, 128], FP16, tag=f"wih{nm}", name=f"wih{nm}")
            nc.sync.dma_start(wih[nm][:], wih_d[nm][:].rearrange("kc p mc c -> p kc mc c"))
            whh[nm] = rec.tile([128, 2, 8, 128], FP16, tag=f"whh{nm}", name=f"whh{nm}")
            nc.sync.dma_start(whh[nm][:], whh_d[nm][:].rearrange("kc p mc c -> p kc mc c"))

        # history tensors (fp16, hold h' = 2h)
        rawT = persist.tile([128, 4, BC, S], FP16, tag="rawT", name="rawT")
        sumT = persist.tile([128, 4, BC, SS], FP16, tag="sumT", name="sumT")

        # ---- group state ----
        groups = []
        for gi, (base, Kg, L) in enumerate(RAW_GROUPS):
            groups.append(Group(gi, base, Kg, L, ("rf", "rb"), S, False))
        for gi, (base, Kg, L) in enumerate(SUM_GROUPS):
            groups.append(Group(len(RAW_GROUPS) + gi, base, Kg, L,
                                ("sf", "sb"), SS, True))

        xgw = {}
        C, hs = {}, {}
        for g in groups:
            C[g.id] = rec.tile([128, 2, 2, g.K, BC], F32, tag=f"C{g.id}",
                               name=f"C{g.id}")
            nc.vector.memset(C[g.id][:], 0.0)
            hs[g.id] = []
            for par in range(2):
                t = rec.tile([128, 2, 2, g.K, BC], FP16, tag=f"hs{g.id}_{par}",
                             name=f"hs{g.id}_{par}")
                hs[g.id].append(t)
            if g.xwin:
                xgw[g.id] = [rec.tile([128, 2, g.xwin, 8, g.K * BC], FP16,
                                      tag=f"xgw{g.id}_{pb}", name=f"xgw{g.id}_{pb}")
                             for pb in range(2)]

        def xcol0(g, rr, d):
            # x column (into padded buffer) for segment 0 at round rr
            if d == 0:
                return PAD + g.base + (rr - W)
            return PAD + (g.Sg - g.base) + (W - 1) - rr

        def hist_t0(g, rstep, d):
            # history t for segment 0 at real step rstep
            if d == 0:
                return g.base + rstep
            return g.Sg - 1 - g.base - rstep

        evac_i = [0]

        def emit_round(g, rr):
            xsb = xs_sb if g.is_sum else xr_sb
            XL = XS if g.is_sum else XR
            hist = sumT if g.is_sum else rawT
            Sg, L, Kg = g.Sg, g.L, g.K
            if g.xwin and rr % g.xwin == 0:
                # batch x-projection for rounds [rr, rr+xwin) into SBUF window
                wn = min(g.xwin, g.rounds - rr)
                wt = xgw[g.id][(rr // g.xwin) % 2]
                for d in (0, 1):
                    st = L if d == 0 else -L
                    wi = wih[g.dirsets[d]]
                    # fwd: ascending cols from rr; bwd: window cols ascending
                    # from the round rr+wn-1 position (rw index reversed)
                    cb = xcol0(g, rr, d) if d == 0 else xcol0(g, rr + wn - 1, d)
                    for mc in range(8):
                        pb = xbp.tile([128, Kg, BC, g.xwin], F32, tag="xb",
                                      name="xb")
                        for kc in range(3):
                            mv = _tap(xsb, kc * (BC * XL) + cb,
                                      [[st, Kg], [XL, BC], [1, wn]])
                            nc.tensor.matmul(pb[:, :, :, :wn], wi[:, kc, mc, :],
                                             mv, start=(kc == 0), stop=(kc == 2))
                        # evac psum [K,BC,win] -> window [win, mc, K*BC]
                        dstap = _tap(wt, d * (g.xwin * 8 * Kg * BC) + mc * Kg * BC,
                                     [[8 * Kg * BC, wn], [1, Kg * BC]])
                        srcap = _tap(pb, 0, [[1, wn], [g.xwin, Kg * BC]])
                        if evac_i[0] % 2 == 0:
                            nc.vector.tensor_copy(dstap, srcap)
                        else:
                            nc.scalar.copy(dstap, srcap)
                        evac_i[0] += 1
            ps = gps.tile([128, 2, 8, Kg, BC], F32, tag=f"ps{g.id}", name=f"ps{g.id}")
            for d in (0, 1):
                st = L if d == 0 else -L
                c0 = xcol0(g, rr, d)
                wi, wh = wih[g.dirsets[d]], whh[g.dirsets[d]]
                if g.xwin:
                    wt = xgw[g.id][(rr // g.xwin) % 2]
                    wn = min(g.xwin, g.rounds - (rr - rr % g.xwin))
                    rw = rr % g.xwin if d == 0 else wn - 1 - (rr % g.xwin)
                    nc.tensor.matmul(
                        ps[:, d, :, :, :], identh[:],
                        wt[:, d, rw, :, :], start=True,
                        stop=(rr == 0), skip_group_check=True)
                for mc in range(8):
                    o = ps[:, d, mc, :, :]
                    if not g.xwin:
                        for kc in range(3):
                            mv = _tap(xsb, kc * (BC * XL) + c0, [[st, Kg], [XL, BC]])
                            nc.tensor.matmul(o, wi[:, kc, mc, :], mv,
                                             start=(kc == 0),
                                             stop=(kc == 2 and rr == 0))
                    if rr > 0:
                        for kc in range(2):
                            if rr - 1 < W:
                                hm = hs[g.id][(rr - 1) % 2][:, d, kc, :, :]
                            else:
                                t0p = hist_t0(g, rr - 1 - W, d)
                                hm = _tap(hist, (2 * d + kc) * BC * Sg + t0p,
                                          [[st, Kg], [Sg, BC]])
                            nc.tensor.matmul(o, wh[:, kc, mc, :], hm,
                                             start=False, stop=(kc == 1),
                                             skip_group_check=g.xwin is not None)
            th = work.tile([128, 2, 8, Kg, BC], FP16, tag=f"th{g.id}", name=f"th{g.id}")
            nc.scalar.activation(th[:], ps[:], AF.Tanh)
            # state D = 2C:  D' = (tf+1)*0.5*D + (ti+1)*tg ; tc = tanh(0.5 D')
            ths = {blk: th[:, :, 2 * blk:2 * blk + 2, :, :]
                   .rearrange("p d m k b -> p d m (k b)") for blk in range(4)}
            Dap = C[g.id][:].rearrange("p d m k b -> p d m (k b)")
            u = work.tile([128, 2, 2, Kg * BC], F32, tag=f"u{g.id}", name=f"u{g.id}")
            nc.vector.scalar_tensor_tensor(out=u[:], in0=ths[0], scalar=1.0,
                                           in1=ths[2], op0=ALU.add, op1=ALU.mult)
            v = work.tile([128, 2, 2, Kg * BC], F32, tag=f"v{g.id}", name=f"v{g.id}")
            nc.vector.scalar_tensor_tensor(out=v[:], in0=ths[1], scalar=1.0,
                                           in1=Dap, op0=ALU.add, op1=ALU.mult)
            nc.vector.scalar_tensor_tensor(out=Dap, in0=v[:], scalar=0.5,
                                           in1=u[:], op0=ALU.mult, op1=ALU.add)
            tcl = work.tile([128, 2, 2, Kg, BC], FP16, tag=f"tc{g.id}", name=f"tc{g.id}")
            nc.scalar.activation(tcl[:], C[g.id][:], AF.Tanh, scale=0.5)
            for d in (0, 1):
                eng = nc.vector
                for kc in range(2):
                    if rr < W:
                        dst = hs[g.id][rr % 2][:, d, kc, :, :]
                    else:
                        st = L if d == 0 else -L
                        t0 = hist_t0(g, rr - W, d)
                        dst = _tap(hist, (2 * d + kc) * BC * Sg + t0,
                                   [[st, Kg], [Sg, BC]])
                    eng.scalar_tensor_tensor(
                        out=dst, in0=th[:, d, 6 + kc, :, :], scalar=1.0,
                        in1=tcl[:, d, kc, :, :], op0=ALU.add, op1=ALU.mult)
            if rr == W - 1 and g.base == 0:
                # segment 0 (fwd [0,L), bwd [Sg-L,Sg)) starts from the true
                # zero state: discard its garbage warmup state.
                nc.vector.memset(C[g.id][:, :, :, 0, :], 0.0)
                nc.vector.memset(hs[g.id][(W - 1) % 2][:, :, :, 0, :], 0.0)

        # phase D tiles that later phases need
        kT_r = persist.tile([128, NH, BC], FP16, tag="kT_r", name="kT_r")
        v1 = persist.tile([1, BC, NH, DH], FP16, tag="v1", name="v1")
        wt_r = persist.tile([128, NH, 4, BC], FP16, tag="wt_r", name="wt_r")

        def emit_phase_d():
            with tc.tile_pool(name="dpool", bufs=1) as pl, \
                 tc.tile_pool(name="d_ps", bufs=1, space="PSUM") as dps:
                msk = pl.tile([128, 4, BC, SS], FP16, tag="msk", name="msk")
                srcap = bass.AP(tensor=maskdiv_d, offset=0,
                                ap=[[0, 128], [SS, BC], [1, SS]])
                for dk in range(4):
                    nc.sync.dma_start(msk[:, dk, :, :], srcap)
                masked = pl.tile([128, 4, BC, SS], FP16, tag="masked", name="masked")
                nc.vector.tensor_tensor(out=masked[:], in0=sumT[:], in1=msk[:],
                                        op=ALU.mult)
                sv = pl.tile([128, 4, BC], F32, tag="sv", name="sv")
                nc.vector.tensor_reduce(out=sv[:], in_=masked[:],
                                        axis=mybir.AxisListType.X, op=ALU.add)
                sv_h = pl.tile([128, 4, BC], FP16, tag="sv_h", name="sv_h")
                nc.vector.tensor_copy(sv_h[:], sv[:])

                wkv = pl.tile([128, 2, NH, 4, DH], FP16, tag="wkv", name="wkv")
                nc.sync.dma_start(wkv[:], wkv_d[:])
                ps_kv = dps.tile([128, NH, 2, BC], F32, tag="dps", name="ps_kv")
                for h in range(NH):
                    for ih in range(2):
                        for dk in range(4):
                            nc.tensor.matmul(ps_kv[:, h, ih, :],
                                             wkv[:, ih, h, dk, :],
                                             sv_h[:, dk, :],
                                             start=(dk == 0), stop=(dk == 3))
                nc.vector.tensor_copy(kT_r[:], ps_kv[:, :, 0, :])
                v_sb = pl.tile([128, NH, BC], FP16, tag="v_sb", name="v_sb")
                nc.scalar.copy(v_sb[:], ps_kv[:, :, 1, :])
                ps_vt = dps.tile([BC, NH, DH], FP16, tag="dps", name="ps_vt")
                for h in range(NH):
                    nc.tensor.transpose(ps_vt[:, h, :], v_sb[:, h, :], identh[:])
                v4 = pl.tile([BC, NH, DH], FP16, tag="v4", name="v4")
                nc.vector.tensor_copy(v4[:], ps_vt[:])
                for b in range(BC):
                    nc.sync.dma_start(v1[:, b, :, :], v4[b:b + 1, :, :])

                # w_tilde[dmod, h, dk, b] = sum_e (0.5 Wq)[h][dk*128+dmod, e] k[e, h, b]
                wqtt = pl.tile([128, NH, 4, DH], FP16, tag="wqtt", name="wqtt")
                nc.sync.dma_start(wqtt[:], wqtt_d[:])
                ps_wt = dps.tile([128, NH, 4, BC], F32, tag="dps", name="ps_wt")
                for h in range(NH):
                    for dk in range(4):
                        nc.tensor.matmul(ps_wt[:, h, dk, :], wqtt[:, h, dk, :],
                                         kT_r[:, h, :], start=True, stop=True)
                nc.vector.tensor_copy(wt_r[:], ps_wt[:])

        # ================= recurrence rotation =================
        max_rounds = max(g.rounds for g in groups)
        sum_last = max(g.rounds for g in groups if g.is_sum) - 1
        for rr in range(max_rounds):
            for g in groups:
                if rr < g.rounds:
                    emit_round(g, rr)
            if rr == sum_last:
                emit_phase_d()

        xbp.release()
        gps.release()
        work.release()
        rec.release()

        # ================= phase E: scores, softmax, rst =================
        with tc.tile_pool(name="att", bufs=1) as att, \
             tc.tile_pool(name="attw", bufs=2) as attw, \
             tc.tile_pool(name="sc_ps", bufs=3, space="PSUM") as sc_ps, \
             tc.tile_pool(name="r_ps", bufs=3, space="PSUM") as r_ps, \
             tc.tile_pool(name="t_ps", bufs=2, space="PSUM") as t_ps:
            wq_r = att.tile([128, 4, NH, DH], FP16, tag="wq_r", name="wq_r")
            nc.sync.dma_start(wq_r[:], wqst_d[:].rearrange("dk p h e -> p dk h e"))

            # scores[b*4+h, s] = sum_d rawT[d, b, s] * wt[d, h, b]
            scores_sb = att.tile([16, S], F32, tag="scores", name="scores")
            ei = 0
            for b in range(BC):
                for half in range(2):
                    sl = slice(half * 512, (half + 1) * 512)
                    pssc = sc_ps.tile([4, 512], F32, tag="sc", name="sc")
                    for dk in range(4):
                        nc.tensor.matmul(pssc[:], wt_r[:, :, dk, b],
                                         rawT[:, dk, b, sl],
                                         start=(dk == 0), stop=(dk == 3))
                    s4 = attw.tile([4, 512], F32, tag="s4", name="s4", bufs=3)
                    if ei % 2 == 0:
                        nc.scalar.copy(s4[:], pssc[:])
                    else:
                        nc.vector.tensor_copy(s4[:], pssc[:])
                    nc.sync.dma_start(scores_sb[b * 4:(b + 1) * 4, sl], s4[:])
                    ei += 1
            rmax = attw.tile([16, 1], F32, tag="rmax", name="rmax")
            nc.vector.tensor_reduce(out=rmax[:], in_=scores_sb[:],
                                    axis=mybir.AxisListType.X, op=ALU.max)
            nmax = attw.tile([16, 1], F32, tag="nmax", name="nmax")
            nc.vector.tensor_scalar_mul(nmax[:], rmax[:], -1.0)
            e_sb = attw.tile([16, S], F32, tag="e_sb", name="e_sb", bufs=1)
            nc.scalar.activation(e_sb[:], scores_sb[:], AF.Exp, bias=nmax[:],
                                 scale=1.0)
            zs = attw.tile([16, 1], F32, tag="zs", name="zs")
            nc.vector.tensor_reduce(out=zs[:], in_=e_sb[:],
                                    axis=mybir.AxisListType.X, op=ALU.add)
            rz = attw.tile([16, 1], F32, tag="rz", name="rz")
            nc.vector.reciprocal(rz[:], zs[:])
            attn = att.tile([16, S], FP16, tag="attn", name="attn")
            nc.vector.tensor_scalar_mul(attn[:], e_sb[:], rz[:])

            # rst[e, s] = sum_dk wq[dk, h, e] rawT[dk, b, s] + v[e] attn[s]
            ei = 0
            for b in range(BC):
                rstT = attw.tile([128, NH, S], FP16, tag="rstT", name="rstT")
                for h in range(NH):
                    row = b * 4 + h
                    attn1 = attw.tile([1, S], FP16, tag="attn1", name="attn1",
                                      bufs=3)
                    nc.sync.dma_start(attn1[:], attn[row:row + 1, :])
                    for half in range(2):
                        sl = slice(half * 512, (half + 1) * 512)
                        psr = r_ps.tile([128, 512], F32, tag="rst", name="rst")
                        for dk in range(4):
                            nc.tensor.matmul(psr[:], wq_r[:, dk, h, :],
                                             rawT[:, dk, b, sl],
                                             start=(dk == 0), stop=False)
                        nc.tensor.matmul(psr[:], v1[:, b, h, :],
                                         attn1[:, sl],
                                         start=False, stop=True)
                        if ei % 2 == 0:
                            nc.scalar.copy(rstT[:, h, sl], psr[:])
                        else:
                            nc.vector.tensor_copy(rstT[:, h, sl], psr[:])
                        ei += 1
                for tch in range(8):
                    obuf = attw.tile([128, NH, DH], F32, tag="obuf", name="obuf",
                                     bufs=3)
                    for h in range(NH):
                        ps_t = t_ps.tile([128, DH], FP16, tag="ps_t", name="ps_t")
                        nc.tensor.transpose(
                            ps_t[:], rstT[:, h, tch * 128:(tch + 1) * 128],
                            identh[:])
                        if h % 2 == 0:
                            nc.scalar.copy(obuf[:, h, :], ps_t[:])
                        else:
                            nc.vector.tensor_copy(obuf[:, h, :], ps_t[:])
                    nc.sync.dma_start(
                        out_d[b, tch * 128:(tch + 1) * 128, :],
                        obuf[:].rearrange("p h e -> p (h e)"))

        persist.release()

    nc.compile()
    return nc


# ======================= host-side prep =======================

_GS = np.concatenate([np.full(2 * H, 0.5, np.float32),
                      np.full(H, 1.0, np.float32),
                      np.full(H, 0.5, np.float32)])  # i,f,o tanh-trick scaling


def _prep_shared(inputs):
    sh = {}
    for nm, pre in [("rf", "raw_f"), ("rb", "raw_b"), ("sf", "sum_f"), ("sb", "sum_b")]:
        wihm = np.asarray(inputs[pre + "_Wih"], np.float32)   # [1024, 300]
        bb = np.asarray(inputs[pre + "_b"], np.float32)
        whhm = np.asarray(inputs[pre + "_Whh"], np.float32)   # [1024, 256]
        aug = np.zeros((384, 4 * H), np.float32)
        aug[:D_IN] = wihm.T
        aug[D_IN] = bb
        aug *= _GS[None, :]
        sh[f"wih_{nm}"] = np.ascontiguousarray(
            aug.astype(ml_dtypes.bfloat16).reshape(3, 128, 8, 128))
        whhT = (whhm.T * _GS[None, :] * 0.5).astype(ml_dtypes.bfloat16)
        sh[f"whh_{nm}"] = np.ascontiguousarray(whhT.reshape(2, 128, 8, 128))
    wq = np.asarray(inputs["Wq"], np.float32) * 0.5          # [NH, 512, 128]
    wq4 = wq.reshape(NH, 4, 128, DH)
    sh["wqst"] = np.ascontiguousarray(
        np.transpose(wq4, (1, 2, 0, 3)).astype(ml_dtypes.bfloat16))
    sh["wqtt"] = np.ascontiguousarray(
        np.transpose(wq4, (3, 0, 1, 2)).astype(ml_dtypes.bfloat16))
    wk4 = np.asarray(inputs["Wk"], np.float32).reshape(NH, 4, 128, DH)
    wv4 = np.asarray(inputs["Wv"], np.float32).reshape(NH, 4, 128, DH)
    sh["wkv"] = np.ascontiguousarray(
        np.stack([np.transpose(wk4, (2, 0, 1, 3)),
                  np.transpose(wv4, (2, 0, 1, 3))], axis=1).astype(ml_dtypes.bfloat16))
    return sh


def _prep_core_inputs(c, inputs, shared):
    rows = slice(c * BC, (c + 1) * BC)
    m = dict(shared)
    xr = np.zeros((384, BC, XR), np.float32)
    xr[:D_IN, :, PAD:PAD + S] = np.transpose(np.asarray(inputs["in_raw"][rows],
                                                        np.float32), (2, 0, 1))
    xr[D_IN] = 1.0
    m["xr"] = np.ascontiguousarray(
        xr.astype(ml_dtypes.bfloat16).reshape(3, 128, BC, XR))
    xsv = np.zeros((384, BC, XS), np.float32)
    xsv[:D_IN, :, PAD:PAD + SS] = np.transpose(np.asarray(inputs["in_sum"][rows],
                                                          np.float32), (2, 0, 1))
    xsv[D_IN] = 1.0
    m["xs"] = np.ascontiguousarray(
        xsv.astype(ml_dtypes.bfloat16).reshape(3, 128, BC, XS))
    lens = np.asarray(inputs["len_sum"][rows])
    mask = (np.arange(SS)[None, :] < lens[:, None]).astype(np.float32)
    m["maskdiv"] = np.ascontiguousarray(
        (mask * 0.5 / np.maximum(lens, 1).astype(np.float32)[:, None])
        .astype(ml_dtypes.bfloat16))
    return m


_NC_CACHE = {}


def get_nc():
    if "nc" not in _NC_CACHE:
        _NC_CACHE["nc"] = build_nc()
    return _NC_CACHE["nc"]


def kernel(**inputs) -> np.ndarray:
    nc = get_nc()
    shared = _prep_shared(inputs)
    in_maps = [_prep_core_inputs(c, inputs, shared) for c in range(NCORES)]
    trace = bool(int(os.environ.get("K_TRACE", "0")))
    res = bass_utils.run_bass_kernel_spmd(
        nc, in_maps, core_ids=list(range(NCORES)), trace=trace)
    if trace and res.exec_time_ns is not None:
        print(f"HW exec time: {res.exec_time_ns} ns")
        kernel.last_exec_ns = res.exec_time_ns
    kernel.last_results = res
    out = np.concatenate([res.results[c]["out"] for c in range(NCORES)], axis=0)
    return out


# revision 19
# speedup vs baseline: 1.8306x; 1.1913x over previous
"""Trainium2 Bass kernel for nn_BiLSTM_centric_layer — segmented-recurrence design.

Key ideas vs the naive data-parallel kernel:

1. The LSTM recurrence is LATENCY-bound on TRN2: each chain's loop-carried
   dependency (matmul -> tanh(gates) -> cell update -> tanh(C) -> h) costs
   ~2-3us of semaphore/engine-init latency per step, and plain BiLSTM only
   exposes 2 independent chains (fwd/bwd). We exploit exponential LSTM state
   forgetting (prod of sigmoid(f) decays ~e^-0.45/step) to split the sequence
   into P segments, each warmed up W steps from zero state (numerically
   validated: W=16 -> ~3e-4 state error, decaying further downstream).
   This gives 2P independent chains.

2. K=4 segments advance in lockstep inside single fat instructions (one
   tanh over all of them), amortizing the ~370ns fixed cost of each
   Activation-engine op; R=5 such groups rotate to hide the dependency
   latency.

3. Input-gate terms x@Wih are accumulated straight into the gates PSUM tile
   by the PE (3 extra tiny matmuls per gate chunk) — no xg precompute pass,
   no DRAM round trip, no vector-engine add.

4. h' = 2h = (tanh_o + 1) * tanh(C) is ONE fused DVE op
   (scalar_tensor_tensor); the 0.5 factors are folded into Whh, Wq and the
   mean-pool mask on the host. h' is written fp16 directly into the SBUF
   history tensor which the next step's matmul reads in place.

5. Attention scores are computed directly from the history via
   w_tilde = (0.5 Wq) @ k (rank-1), so q is only materialized inside the
   fused q + attn (x) v PSUM accumulation.

Sharding: data-parallel over batch (4 rows/core x 8 cores), weights
replicated. Everything hardcoded for B=32, S_RAW=1024, S_SUM=128, D_IN=300,
H=256, NH=4.
"""
import os
import sys

sys.path.insert(0, "/opt/trn_rl_repo")

import numpy as np
import ml_dtypes

import concourse.bacc as bacc
import concourse.bass as bass
import concourse.mybir as mybir
import concourse.tile as tile
from concourse import bass_utils
from concourse.masks import make_identity

F32 = mybir.dt.float32
F32R = mybir.dt.float32r
FP16 = mybir.dt.bfloat16  # bf16: fast PE path
AF = mybir.ActivationFunctionType
ALU = mybir.AluOpType

B, S, SS, D_IN, H, NH = 32, 1024, 128, 300, 256, 4
DH = 128
BC = 4             # batch rows per core
NCORES = 8
PAD = 32           # zero padding (timesteps) on both ends of x
XR = S + 2 * PAD   # padded raw length
XS = SS + 2 * PAD  # padded sum length
W = int(os.environ.get("K_W", "12"))            # warmup steps
assert 1 <= W <= PAD

# rotation groups: (base, K segments, seg_len) covering [base, base+K*L);
# fwd seg j = [base+j*L, ...); bwd mirrored. Fat K amortizes the ~36ns
# per-matmul weight-load cost; >=2 raw groups hide the chain latency.
RAW_GROUPS = [(0, 16, 32), (512, 16, 32)]
assert sum(Kg * L for _, Kg, L in RAW_GROUPS) == S
SUM_GROUPS = [(0, 8, 16)]
assert sum(Kg * L for _, Kg, L in SUM_GROUPS) == SS
XWIN = 4           # x-projection batch window (rounds) for raw groups


class Group:
    def __init__(self, gid, base, Kg, L, dirsets, Sg, is_sum):
        self.id, self.base, self.K, self.L = gid, base, Kg, L
        self.dirsets, self.Sg, self.is_sum = dirsets, Sg, is_sum
        self.rounds = W + L
        self.xwin = None if is_sum else XWIN


def _tap(t, off, dims):
    full = t[:]
    return bass.AP(tensor=full.tensor, offset=full.offset + off,
                   ap=[list(full.ap[0])] + [list(d) for d in dims])


def build_nc():
    nc = bacc.Bacc("TRN2", target_bir_lowering=False, debug=False)

    # ---- DRAM I/O ----
    xr_d = nc.dram_tensor("xr", [3, 128, BC, XR], FP16, kind="ExternalInput")
    xs_d = nc.dram_tensor("xs", [3, 128, BC, XS], FP16, kind="ExternalInput")
    wih_d, whh_d = {}, {}
    for nm in ["rf", "rb", "sf", "sb"]:
        wih_d[nm] = nc.dram_tensor(f"wih_{nm}", [3, 128, 8, 128], FP16,
                                   kind="ExternalInput")
        whh_d[nm] = nc.dram_tensor(f"whh_{nm}", [2, 128, 8, 128], FP16,
                                   kind="ExternalInput")
    wqst_d = nc.dram_tensor("wqst", [4, 128, NH, DH], FP16, kind="ExternalInput")
    wqtt_d = nc.dram_tensor("wqtt", [128, NH, 4, DH], FP16, kind="ExternalInput")
    wkv_d = nc.dram_tensor("wkv", [128, 2, NH, 4, DH], FP16, kind="ExternalInput")
    maskdiv_d = nc.dram_tensor("maskdiv", [BC, SS], FP16, kind="ExternalInput")
    out_d = nc.dram_tensor("out", [BC, S, NH * DH], F32, kind="ExternalOutput")

    with tile.TileContext(nc) as tc:
        persist = tc.alloc_tile_pool(name="persist", bufs=1)
        rec = tc.alloc_tile_pool(name="rec", bufs=1)        # released before E
        work = tc.alloc_tile_pool(name="work", bufs=2)
        gps = tc.alloc_tile_pool(name="gps", bufs=1, space="PSUM")
        xbp = tc.alloc_tile_pool(name="xbp", bufs=2, space="PSUM")

        ident = persist.tile([128, 128], F32, tag="ident", name="ident")
        make_identity(nc, ident[:])
        identh = persist.tile([128, 128], FP16, tag="identh", name="identh")
        nc.vector.tensor_copy(identh[:], ident[:])

        # ---- stage inputs in SBUF ----
        xr_sb = rec.tile([128, 3, BC, XR], FP16, tag="xr", name="xr_sb")
        nc.sync.dma_start(xr_sb[:], xr_d[:].rearrange("kc p b t -> p kc b t"))
        xs_sb = rec.tile([128, 3, BC, XS], FP16, tag="xs", name="xs_sb")
        nc.sync.dma_start(xs_sb[:], xs_d[:].rearrange("kc p b t -> p kc b t"))
        wih, whh = {}, {}
        for nm in ["rf", "rb", "sf", "sb"]:
            wih[nm] = rec.tile([128, 3, 8, 128], FP16, tag=f"wih{nm}", name=f"wih{nm}")
            nc.sync.dma_start(wih[nm][:], wih_d[nm][:].rearrange("kc p mc c -> p kc mc c"))
            whh[nm] = rec.tile([128, 2, 8, 128], FP16, tag=f"whh{nm}", name=f"whh{nm}")
            nc.sync.dma_start(whh[nm][:], whh_d[nm][:].rearrange("kc p mc c -> p kc mc c"))

        # history tensors (fp16, hold h' = 2h)
        rawT = persist.tile([128, 4, BC, S], FP16, tag="rawT", name="rawT")
        sumT = persist.tile([128, 4, BC, SS], FP16, tag="sumT", name="sumT")

        # ---- group state ----
        groups = []
        for gi, (base, Kg, L) in enumerate(RAW_GROUPS):
            groups.append(Group(gi, base, Kg, L, ("rf", "rb"), S, False))
        for gi, (base, Kg, L) in enumerate(SUM_GROUPS):
            groups.append(Group(len(RAW_GROUPS) + gi, base, Kg, L,
                                ("sf", "sb"), SS, True))

        xgw = {}
        C, hs = {}, {}
        for g in groups:
            C[g.id] = rec.tile([128, 2, 2, g.K, BC], F32, tag=f"C{g.id}",
                               name=f"C{g.id}")
            nc.vector.memset(C[g.id][:], 0.0)
            hs[g.id] = []
            for par in range(2):
                t = rec.tile([128, 2, 2, g.K, BC], FP16, tag=f"hs{g.id}_{par}",
                             name=f"hs{g.id}_{par}")
                hs[g.id].append(t)
            if g.xwin:
                xgw[g.id] = [rec.tile([128, 2, g.xwin, 8, g.K * BC], FP16,
                                      tag=f"xgw{g.id}_{pb}", name=f"xgw{g.id}_{pb}")
                             for pb in range(2)]

        def xcol0(g, rr, d):
            # x column (into padded buffer) for segment 0 at round rr
            if d == 0:
                return PAD + g.base + (rr - W)
            return PAD + (g.Sg - g.base) + (W - 1) - rr

        def hist_t0(g, rstep, d):
            # history t for segment 0 at real step rstep
            if d == 0:
                return g.base + rstep
            return g.Sg - 1 - g.base - rstep

        evac_i = [0]

        def emit_round(g, rr):
            xsb = xs_sb if g.is_sum else xr_sb
            XL = XS if g.is_sum else XR
            hist = sumT if g.is_sum else rawT
            Sg, L, Kg = g.Sg, g.L, g.K
            if g.xwin and rr % g.xwin == 0:
                # batch x-projection for rounds [rr, rr+xwin) into SBUF window
                wn = min(g.xwin, g.rounds - rr)
                wt = xgw[g.id][(rr // g.xwin) % 2]
                for d in (0, 1):
                    st = L if d == 0 else -L
                    wi = wih[g.dirsets[d]]
                    # fwd: ascending cols from rr; bwd: window cols ascending
                    # from the round rr+wn-1 position (rw index reversed)
                    cb = xcol0(g, rr, d) if d == 0 else xcol0(g, rr + wn - 1, d)
                    for mc in range(8):
                        pb = xbp.tile([128, Kg, BC, g.xwin], F32, tag="xb",
                                      name="xb")
                        for kc in range(3):
                            mv = _tap(xsb, kc * (BC * XL) + cb,
                                      [[st, Kg], [XL, BC], [1, wn]])
                            nc.tensor.matmul(pb[:, :, :, :wn], wi[:, kc, mc, :],
                                             mv, start=(kc == 0), stop=(kc == 2))
                        # evac psum [K,BC,win] -> window [win, mc, K*BC]
                        dstap = _tap(wt, d * (g.xwin * 8 * Kg * BC) + mc * Kg * BC,
                                     [[8 * Kg * BC, wn], [1, Kg * BC]])
                        srcap = _tap(pb, 0, [[1, wn], [g.xwin, Kg * BC]])
                        if evac_i[0] % 2 == 0:
                            nc.vector.tensor_copy(dstap, srcap)
                        else:
                            nc.scalar.copy(dstap, srcap)
                        evac_i[0] += 1
            ps = gps.tile([128, 2, 8, Kg, BC], F32, tag=f"ps{g.id}", name=f"ps{g.id}")
            for d in (0, 1):
                st = L if d == 0 else -L
                c0 = xcol0(g, rr, d)
                wi, wh = wih[g.dirsets[d]], whh[g.dirsets[d]]
                if g.xwin:
                    wt = xgw[g.id][(rr // g.xwin) % 2]
                    wn = min(g.xwin, g.rounds - (rr - rr % g.xwin))
                    rw = rr % g.xwin if d == 0 else wn - 1 - (rr % g.xwin)
                    nc.tensor.matmul(
                        ps[:, d, :, :, :], identh[:],
                        wt[:, d, rw, :, :], start=True,
                        stop=(rr == 0), skip_group_check=True)
                for mc in range(8):
                    o = ps[:, d, mc, :, :]
                    if not g.xwin:
                        for kc in range(3):
                            mv = _tap(xsb, kc * (BC * XL) + c0, [[st, Kg], [XL, BC]])
                            nc.tensor.matmul(o, wi[:, kc, mc, :], mv,
                                             start=(kc == 0),
                                             stop=(kc == 2 and rr == 0))
                    if rr > 0:
                        for kc in range(2):
                            if rr - 1 < W:
                                hm = hs[g.id][(rr - 1) % 2][:, d, kc, :, :]
                            else:
                                t0p = hist_t0(g, rr - 1 - W, d)
                                hm = _tap(hist, (2 * d + kc) * BC * Sg + t0p,
                                          [[st, Kg], [Sg, BC]])
                            nc.tensor.matmul(o, wh[:, kc, mc, :], hm,
                                             start=False, stop=(kc == 1),
                                             skip_group_check=g.xwin is not None)
            th = work.tile([128, 2, 8, Kg, BC], FP16, tag=f"th{g.id}", name=f"th{g.id}")
            nc.scalar.activation(th[:], ps[:], AF.Tanh)
            # state D = 2C:  D' = (tf+1)*0.5*D + (ti+1)*tg ; tc = tanh(0.5 D')
            ths = {blk: th[:, :, 2 * blk:2 * blk + 2, :, :]
                   .rearrange("p d m k b -> p d m (k b)") for blk in range(4)}
            Dap = C[g.id][:].rearrange("p d m k b -> p d m (k b)")
            u = work.tile([128, 2, 2, Kg * BC], F32, tag=f"u{g.id}", name=f"u{g.id}")
            nc.vector.scalar_tensor_tensor(out=u[:], in0=ths[0], scalar=1.0,
                                           in1=ths[2], op0=ALU.add, op1=ALU.mult)
            v = work.tile([128, 2, 2, Kg * BC], F32, tag=f"v{g.id}", name=f"v{g.id}")
            nc.vector.scalar_tensor_tensor(out=v[:], in0=ths[1], scalar=1.0,
                                           in1=Dap, op0=ALU.add, op1=ALU.mult)
            nc.vector.scalar_tensor_tensor(out=Dap, in0=v[:], scalar=0.5,
                                           in1=u[:], op0=ALU.mult, op1=ALU.add)
            tcl = work.tile([128, 2, 2, Kg, BC], FP16, tag=f"tc{g.id}", name=f"tc{g.id}")
            nc.scalar.activation(tcl[:], C[g.id][:], AF.Tanh, scale=0.5)
            for d in (0, 1):
                eng = nc.vector
                for kc in range(2):
                    if rr < W:
                        dst = hs[g.id][rr % 2][:, d, kc, :, :]
                    else:
                        st = L if d == 0 else -L
                        t0 = hist_t0(g, rr - W, d)
                        dst = _tap(hist, (2 * d + kc) * BC * Sg + t0,
                                   [[st, Kg], [Sg, BC]])
                    eng.scalar_tensor_tensor(
                        out=dst, in0=th[:, d, 6 + kc, :, :], scalar=1.0,
                        in1=tcl[:, d, kc, :, :], op0=ALU.add, op1=ALU.mult)
            if rr == W - 1 and g.base == 0:
                # segment 0 (fwd [0,L), bwd [Sg-L,Sg)) starts from the true
                # zero state: discard its garbage warmup state.
                nc.vector.memset(C[g.id][:, :, :, 0, :], 0.0)
                nc.vector.memset(hs[g.id][(W - 1) % 2][:, :, :, 0, :], 0.0)

        # phase D tiles that later phases need
        kT_r = persist.tile([128, NH, BC], FP16, tag="kT_r", name="kT_r")
        v1 = persist.tile([1, BC, NH, DH], FP16, tag="v1", name="v1")
        wt_r = persist.tile([128, NH, 4, BC], FP16, tag="wt_r", name="wt_r")

        def emit_phase_d():
            with tc.tile_pool(name="dpool", bufs=1) as pl, \
                 tc.tile_pool(name="d_ps", bufs=1, space="PSUM") as dps:
                msk = pl.tile([128, 4, BC, SS], FP16, tag="msk", name="msk")
                srcap = bass.AP(tensor=maskdiv_d, offset=0,
                                ap=[[0, 128], [SS, BC], [1, SS]])
                for dk in range(4):
                    nc.sync.dma_start(msk[:, dk, :, :], srcap)
                masked = pl.tile([128, 4, BC, SS], FP16, tag="masked", name="masked")
                nc.vector.tensor_tensor(out=masked[:], in0=sumT[:], in1=msk[:],
                                        op=ALU.mult)
                sv = pl.tile([128, 4, BC], F32, tag="sv", name="sv")
                nc.vector.tensor_reduce(out=sv[:], in_=masked[:],
                                        axis=mybir.AxisListType.X, op=ALU.add)
                sv_h = pl.tile([128, 4, BC], FP16, tag="sv_h", name="sv_h")
                nc.vector.tensor_copy(sv_h[:], sv[:])

                wkv = pl.tile([128, 2, NH, 4, DH], FP16, tag="wkv", name="wkv")
                nc.sync.dma_start(wkv[:], wkv_d[:])
                ps_kv = dps.tile([128, NH, 2, BC], F32, tag="dps", name="ps_kv")
                for h in range(NH):
                    for ih in range(2):
                        for dk in range(4):
                            nc.tensor.matmul(ps_kv[:, h, ih, :],
                                             wkv[:, ih, h, dk, :],
                                             sv_h[:, dk, :],
                                             start=(dk == 0), stop=(dk == 3))
                nc.vector.tensor_copy(kT_r[:], ps_kv[:, :, 0, :])
                v_sb = pl.tile([128, NH, BC], FP16, tag="v_sb", name="v_sb")
                nc.scalar.copy(v_sb[:], ps_kv[:, :, 1, :])
                ps_vt = dps.tile([BC, NH, DH], FP16, tag="dps", name="ps_vt")
                for h in range(NH):
                    nc.tensor.transpose(ps_vt[:, h, :], v_sb[:, h, :], identh[:])
                v4 = pl.tile([BC, NH, DH], FP16, tag="v4", name="v4")
                nc.vector.tensor_copy(v4[:], ps_vt[:])
                for b in range(BC):
                    nc.sync.dma_start(v1[:, b, :, :], v4[b:b + 1, :, :])

                # w_tilde[dmod, h, dk, b] = sum_e (0.5 Wq)[h][dk*128+dmod, e] k[e, h, b]
                wqtt = pl.tile([128, NH, 4, DH], FP16, tag="wqtt", name="wqtt")
                nc.sync.dma_start(wqtt[:], wqtt_d[:])
                ps_wt = dps.tile([128, NH, 4, BC], F32, tag="dps", name="ps_wt")
                for h in range(NH):
                    for dk in range(4):
                        nc.tensor.matmul(ps_wt[:, h, dk, :], wqtt[:, h, dk, :],
                                         kT_r[:, h, :], start=True, stop=True)
                nc.vector.tensor_copy(wt_r[:], ps_wt[:])

        # ================= recurrence rotation =================
        max_rounds = max(g.rounds for g in groups)
        sum_last = max(g.rounds for g in groups if g.is_sum) - 1
        for rr in range(max_rounds):
            for g in groups:
                if rr < g.rounds:
                    emit_round(g, rr)
            if rr == sum_last:
                emit_phase_d()

        xbp.release()
        gps.release()
        work.release()
        rec.release()

        # ================= phase E: scores, softmax, rst =================
        with tc.tile_pool(name="att", bufs=1) as att, \
             tc.tile_pool(name="attw", bufs=2) as attw, \
             tc.tile_pool(name="sc_ps", bufs=3, space="PSUM") as sc_ps, \
             tc.tile_pool(name="r_ps", bufs=3, space="PSUM") as r_ps, \
             tc.tile_pool(name="t_ps", bufs=2, space="PSUM") as t_ps:
            wq_r = att.tile([128, 4, NH, DH], FP16, tag="wq_r", name="wq_r")
            nc.sync.dma_start(wq_r[:], wqst_d[:].rearrange("dk p h e -> p dk h e"))

            # scores[b*4+h, s] = sum_d rawT[d, b, s] * wt[d, h, b]
            scores_sb = att.tile([16, S], F32, tag="scores", name="scores")
            ei = 0
            for b in range(BC):
                for half in range(2):
                    sl = slice(half * 512, (half + 1) * 512)
                    pssc = sc_ps.tile([4, 512], F32, tag="sc", name="sc")
                    for dk in range(4):
                        nc.tensor.matmul(pssc[:], wt_r[:, :, dk, b],
                                         rawT[:, dk, b, sl],
                                         start=(dk == 0), stop=(dk == 3))
                    s4 = attw.tile([4, 512], F32, tag="s4", name="s4", bufs=3)
                    if ei % 2 == 0:
                        nc.scalar.copy(s4[:], pssc[:])
                    else:
                        nc.vector.tensor_copy(s4[:], pssc[:])
                    nc.sync.dma_start(scores_sb[b * 4:(b + 1) * 4, sl], s4[:])
                    ei += 1
            rmax = attw.tile([16, 1], F32, tag="rmax", name="rmax")
            nc.vector.tensor_reduce(out=rmax[:], in_=scores_sb[:],
                                    axis=mybir.AxisListType.X, op=ALU.max)
            nmax = attw.tile([16, 1], F32, tag="nmax", name="nmax")
            nc.vector.tensor_scalar_mul(nmax[:], rmax[:], -1.0)
            e_sb = attw.tile([16, S], F32, tag="e_sb", name="e_sb", bufs=1)
            nc.scalar.activation(e_sb[:], scores_sb[:], AF.Exp, bias=nmax[:],
                                 scale=1.0)
            zs = attw.tile([16, 1], F32, tag="zs", name="zs")
            nc.vector.tensor_reduce(out=zs[:], in_=e_sb[:],
                                    axis=mybir.AxisListType.X, op=ALU.add)
            rz = attw.tile([16, 1], F32, tag="rz", name="rz")
            nc.vector.reciprocal(rz[:], zs[:])
            attn = att.tile([16, S], FP16, tag="attn", name="attn")
            nc.vector.tensor_scalar_mul(attn[:], e_sb[:], rz[:])

            # rst[e, s] = sum_dk wq[dk, h, e] rawT[dk, b, s] + v[e] attn[s]
            ei = 0
            for b in range(BC):
                rstT = attw.tile([128, NH, S], FP16, tag="rstT", name="rstT")
                for h in range(NH):
                    row = b * 4 + h
                    attn1 = attw.tile([1, S], FP16, tag="attn1", name="attn1",
                                      bufs=3)
                    nc.sync.dma_start(attn1[:], attn[row:row + 1, :])
                    for half in range(2):
                        sl = slice(half * 512, (half + 1) * 512)
                        psr = r_ps.tile([128, 512], F32, tag="rst", name="rst")
                        for dk in range(4):
                            nc.tensor.matmul(psr[:], wq_r[:, dk, h, :],
                                             rawT[:, dk, b, sl],
                                             start=(dk == 0), stop=False)
                        nc.tensor.matmul(psr[:], v1[:, b, h, :],
                                         attn1[:, sl],
                                         start=False, stop=True)
                        if ei % 2 == 0:
                            nc.scalar.copy(rstT[:, h, sl], psr[:])
                        else:
                            nc.vector.tensor_copy(rstT[:, h, sl], psr[:])
                        ei += 1
                for tch in range(8):
                    obuf = attw.tile([128, NH, DH], F32, tag="obuf", name="obuf",
                                     bufs=3)
                    for h in range(NH):
                        ps_t = t_ps.tile([128, DH], FP16, tag="ps_t", name="ps_t")
                        nc.tensor.transpose(
                            ps_t[:], rstT[:, h, tch * 128:(tch + 1) * 128],
                            identh[:])
                        if h % 2 == 0:
                            nc.scalar.copy(obuf[:, h, :], ps_t[:])
                        else:
                            nc.vector.tensor_copy(obuf[:, h, :], ps_t[:])
                    nc.sync.dma_start(
                        out_d[b, tch * 128:(tch + 1) * 128, :],
                        obuf[:].rearrange("p h e -> p (h e)"))

        persist.release()

    nc.compile()
    return nc


# ======================= host-side prep =======================

_GS = np.concatenate([np.full(2 * H, 0.5, np.float32),
                      np.full(H, 1.0, np.float32),
                      np.full(H, 0.5, np.float32)])  # i,f,o tanh-trick scaling


def _prep_shared(inputs):
    sh = {}
    for nm, pre in [("rf", "raw_f"), ("rb", "raw_b"), ("sf", "sum_f"), ("sb", "sum_b")]:
        wihm = np.asarray(inputs[pre + "_Wih"], np.float32)   # [1024, 300]
        bb = np.asarray(inputs[pre + "_b"], np.float32)
        whhm = np.asarray(inputs[pre + "_Whh"], np.float32)   # [1024, 256]
        aug = np.zeros((384, 4 * H), np.float32)
        aug[:D_IN] = wihm.T
        aug[D_IN] = bb
        aug *= _GS[None, :]
        sh[f"wih_{nm}"] = np.ascontiguousarray(
            aug.astype(ml_dtypes.bfloat16).reshape(3, 128, 8, 128))
        whhT = (whhm.T * _GS[None, :] * 0.5).astype(ml_dtypes.bfloat16)
        sh[f"whh_{nm}"] = np.ascontiguousarray(whhT.reshape(2, 128, 8, 128))
    wq = np.asarray(inputs["Wq"], np.float32) * 0.5          # [NH, 512, 128]
    wq4 = wq.reshape(NH, 4, 128, DH)
    sh["wqst"] = np.ascontiguousarray(
        np.transpose(wq4, (1, 2, 0, 3)).astype(ml_dtypes.bfloat16))
    sh["wqtt"] = np.ascontiguousarray(
        np.transpose(wq4, (3, 0, 1, 2)).astype(ml_dtypes.bfloat16))
    wk4 = np.asarray(inputs["Wk"], np.float32).reshape(NH, 4, 128, DH)
    wv4 = np.asarray(inputs["Wv"], np.float32).reshape(NH, 4, 128, DH)
    sh["wkv"] = np.ascontiguousarray(
        np.stack([np.transpose(wk4, (2, 0, 1, 3)),
                  np.transpose(wv4, (2, 0, 1, 3))], axis=1).astype(ml_dtypes.bfloat16))
    return sh


def _prep_core_inputs(c, inputs, shared):
    rows = slice(c * BC, (c + 1) * BC)
    m = dict(shared)
    xr = np.zeros((384, BC, XR), np.float32)
    xr[:D_IN, :, PAD:PAD + S] = np.transpose(np.asarray(inputs["in_raw"][rows],
                                                        np.float32), (2, 0, 1))
    xr[D_IN] = 1.0
    m["xr"] = np.ascontiguousarray(
        xr.astype(ml_dtypes.bfloat16).reshape(3, 128, BC, XR))
    xsv = np.zeros((384, BC, XS), np.float32)
    xsv[:D_IN, :, PAD:PAD + SS] = np.transpose(np.asarray(inputs["in_sum"][rows],
                                                          np.float32), (2, 0, 1))
    xsv[D_IN] = 1.0
    m["xs"] = np.ascontiguousarray(
        xsv.astype(ml_dtypes.bfloat16).reshape(3, 128, BC, XS))
    lens = np.asarray(inputs["len_sum"][rows])
    mask = (np.arange(SS)[None, :] < lens[:, None]).astype(np.float32)
    m["maskdiv"] = np.ascontiguousarray(
        (mask * 0.5 / np.maximum(lens, 1).astype(np.float32)[:, None])
        .astype(ml_dtypes.bfloat16))
    return m


_NC_CACHE = {}


def get_nc():
    if "nc" not in _NC_CACHE:
        _NC_CACHE["nc"] = build_nc()
    return _NC_CACHE["nc"]


def kernel(**inputs) -> np.ndarray:
    nc = get_nc()
    shared = _prep_shared(inputs)
    in_maps = [_prep_core_inputs(c, inputs, shared) for c in range(NCORES)]
    trace = bool(int(os.environ.get("K_TRACE", "0")))
    res = bass_utils.run_bass_kernel_spmd(
        nc, in_maps, core_ids=list(range(NCORES)), trace=trace)
    if trace and res.exec_time_ns is not None:
        print(f"HW exec time: {res.exec_time_ns} ns")
        kernel.last_exec_ns = res.exec_time_ns
    kernel.last_results = res
    out = np.concatenate([res.results[c]["out"] for c in range(NCORES)], axis=0)
    return out
